# revision 6
# baseline (speedup 1.0000x reference)
"""GCN (2-layer, PyG GCNConv semantics) on 8 Trainium2 NeuronCores.

Strategy (dst-shard, graph-parallel):
- Nodes are sharded contiguously across the 8 cores (12500 dsts/core).
- All dense math runs on-device via Bass/Tile in 3 SPMD dispatches:
    A: h1 = x @ W1           (x shipped pre-transposed in bf16, PE matmuls)
    B: s1 = segment-sum of gathered u1 rows over dst groups (PE staircase
       one-hot matmuls built on-device from per-slot dst offsets), fused
       epilogue -> relu1, v2 = dinv*relu1
    C: same segment-sum machinery for layer 2, then @W2 + b2 + log_softmax
- The edge structure (sort order, slot layout, staircase metadata) is
  compile-time constant: it is baked into the instruction stream / tiny
  static inputs at kernel-build time.
- The two per-edge value gathers (u[src] for 3.2M edges) run on the host
  between dispatches: every data-driven gather primitive available in this
  toolchain was measured unusable (indirect DMA ~1.6us/row and 128 rows per
  call; GPSIMD gather ucode unloadable under this walrus build).
"""
import os
import sys
import numpy as np

sys.path.insert(0, "/opt/trn_rl_repo")

import ml_dtypes
import concourse.bass as bass
import concourse.mybir as mybir
import concourse.tile as tile
from concourse.vector_clock import ScopedClock
from concourse.bass_utils import run_bass_kernel_spmd

BF16 = mybir.dt.bfloat16
F32 = mybir.dt.float32
AF = mybir.ActivationFunctionType
ALU = mybir.AluOpType
NPBF16 = ml_dtypes.bfloat16

N_CORES = 8
GROUP = 32          # dsts per staircase group (matmul M)
SC = 64             # chunks per superchunk (is_equal batch)

# ---------------------------------------------------------------------------
# walrus workaround: only ONE sync-wait command per instruction is accepted.
# ---------------------------------------------------------------------------


def _patched_drain_and_barrier(self, tick_clock, wait_clock):
    nc = self.nc
    carrier = nc.sync.nop(nofuse=True, hint="drain_wait_carrier")
    wait_clock.add_sem_waits(carrier.ins, ScopedClock({None: tick_clock.global_clock}))
    si = carrier.ins.sync_info
    waits = list(si.on_wait or []) if si else []
    if len(waits) > 1:
        si.on_wait = waits[:1]
        for i in range(1, len(waits)):
            extra = nc.sync.nop(nofuse=True, hint="drain_wait_carrier")
            extra.ins.sync_info = mybir.SyncInfo(on_wait=waits[i : i + 1], on_update=[])
    nc.sync.drain()
    nc.all_engine_barrier()
    assert self.sems is not None
    popped = nc._tile_sem_poison_stack.pop()
    assert popped is self._sem_poison
    nc.clear_and_free_semaphores(list(self.sems.allocated().values()))
    nc.all_engine_barrier()


tile.TileContext._drain_and_barrier = _patched_drain_and_barrier


def _legalize_waits(nc, max_waits=1):
    n = [0]

    def mk_nop(engine, waits):
        n[0] += 1
        return mybir.InstNoOp(
            name=f"waitnop-{n[0]}",
            engine=engine,
            ins=[],
            outs=[],
            sync_info=mybir.SyncInfo(on_wait=list(waits), on_update=[]),
            text_hint="wait_carrier",
        )

    for f in nc.m.functions:
        for bb in f.blocks:
            out = []
            changed = False
            for inst in bb.instructions:
                si = inst.sync_info
                waits = list(si.on_wait or []) if si else []
                if len(waits) > max_waits:
                    changed = True
                    for i in range(0, len(waits) - max_waits, max_waits):
                        out.append(mk_nop(inst.engine, waits[i : i + max_waits]))
                    si.on_wait = waits[len(waits) - max_waits :]
                out.append(inst)
            if changed:
                bb.instructions = out


# ---------------------------------------------------------------------------
# device kernel builders
# ---------------------------------------------------------------------------


def build_A(NT, FIN=512):
    """h1 = x @ W1 per core. xTr host layout [128, FIN//128, NT*128] bf16."""
    FC = FIN // 128
    nc = bass.Bass()
    xT = nc.dram_tensor("xT", [128, FC, NT * 128], BF16, kind="ExternalInput")
    W1b = nc.dram_tensor("W1b", [128, FC, 16], BF16, kind="ExternalInput")
    h1 = nc.dram_tensor("h1", [NT * 128, 16], F32, kind="ExternalOutput")
    with tile.TileContext(nc) as tc:
        with (
            tc.tile_pool(name="sbuf", bufs=3) as pool,
            tc.tile_pool(name="stat", bufs=1) as spool,
            tc.tile_pool(name="psum", bufs=8, space="PSUM") as pp,
        ):
            w1 = spool.tile([128, FC, 16], BF16)
            nc.sync.dma_start(out=w1[:], in_=W1b[:])
            h_sb = spool.tile([128, NT, 16], F32)
            for t in range(NT):
                xt = pool.tile([128, FC, 128], BF16, tag="xt")
                nc.sync.dma_start(out=xt[:], in_=xT[:, :, 128 * t : 128 * (t + 1)])
                ps = pp.tile([128, 16], F32, tag="hps")
                for fc in range(FC):
                    nc.tensor.matmul(
                        out=ps[:],
                        lhsT=xt[:, fc, :],
                        rhs=w1[:, fc, :],
                        start=(fc == 0),
                        stop=(fc == FC - 1),
                    )
                nc.scalar.copy(out=h_sb[:, t, :], in_=ps[:])
            nc.sync.dma_start(
                out=h1.rearrange("(t p) f -> p t f", p=128), in_=h_sb[:]
            )
    _legalize_waits(nc)
    return nc


def _emit_segsum(nc, tc, pool, spool, pp, g_dram, dstid_sb, iota_sb, chunks, s_sb, nchunks):
    """Staircase segment-sum: s_sb[128, NT, 16] f32 <- sum of g rows per dst."""
    nsc = (nchunks + SC - 1) // SC
    ps = None
    for sc in range(nsc):
        cs = sc * SC
        w = min(SC, nchunks - cs)
        g_sc = pool.tile([128, SC, 16], BF16, tag="gsc")
        nc.sync.dma_start(out=g_sc[:, :w, :], in_=g_dram[:, cs : cs + w, :])
        s_all = pool.tile([128, SC, GROUP], BF16, tag="sall")
        nc.vector.tensor_tensor(
            out=s_all[:, :w, :],
            in0=dstid_sb[:, cs : cs + w].to_broadcast([128, w, GROUP]),
            in1=iota_sb[:, :w, :],
            op=ALU.is_equal,
        )
        for j in range(w):
            grp, st, sp = chunks[cs + j]
            if st:
                ps = pp.tile([GROUP, 16], F32, tag="ps")
            nc.tensor.matmul(
                out=ps[:],
                lhsT=s_all[:, j, :],
                rhs=g_sc[:, j, :],
                start=st,
                stop=sp,
            )
            if sp:
                po = GROUP * (grp % (128 // GROUP))
                nc.scalar.copy(
                    out=s_sb[po : po + GROUP, grp // (128 // GROUP), :], in_=ps[:]
                )


def build_B(NT, nchunks, chunks):
    """s1 -> agg1 -> relu1, v2."""
    nc = bass.Bass()
    g = nc.dram_tensor("g", [128, nchunks, 16], BF16, kind="ExternalInput")
    dstid = nc.dram_tensor("dstid", [128, nchunks], BF16, kind="ExternalInput")
    iota = nc.dram_tensor("iota", [128, SC, GROUP], BF16, kind="ExternalInput")
    h1 = nc.dram_tensor("h1", [NT * 128, 16], F32, kind="ExternalInput")
    dinva = nc.dram_tensor("dinva", [128, NT], F32, kind="ExternalInput")
    dinv2a = nc.dram_tensor("dinv2a", [128, NT], F32, kind="ExternalInput")
    b1rep = nc.dram_tensor("b1rep", [128, NT, 16], F32, kind="ExternalInput")
    relu1 = nc.dram_tensor("relu1", [NT * 128, 16], F32, kind="ExternalOutput")
    v2 = nc.dram_tensor("v2", [NT * 128, 16], BF16, kind="ExternalOutput")
    with tile.TileContext(nc) as tc:
        with (
            tc.tile_pool(name="sbuf", bufs=2) as pool,
            tc.tile_pool(name="stat", bufs=1) as spool,
            tc.tile_pool(name="psum", bufs=8, space="PSUM") as pp,
        ):
            dstid_sb = spool.tile([128, nchunks], BF16)
            nc.sync.dma_start(out=dstid_sb[:], in_=dstid[:])
            iota_sb = spool.tile([128, SC, GROUP], BF16)
            nc.sync.dma_start(out=iota_sb[:], in_=iota[:])
            h1_sb = spool.tile([128, NT, 16], F32)
            nc.sync.dma_start(out=h1_sb[:], in_=h1.rearrange("(t p) f -> p t f", p=128))
            dinva_sb = spool.tile([128, NT], F32)
            nc.sync.dma_start(out=dinva_sb[:], in_=dinva[:])
            dinv2a_sb = spool.tile([128, NT], F32)
            nc.sync.dma_start(out=dinv2a_sb[:], in_=dinv2a[:])
            b1_sb = spool.tile([128, NT, 16], F32)
            nc.sync.dma_start(out=b1_sb[:], in_=b1rep[:])
            s_sb = spool.tile([128, NT, 16], F32)

            _emit_segsum(nc, tc, pool, spool, pp, g, dstid_sb, iota_sb, chunks, s_sb, nchunks)

            tmp = spool.tile([128, NT, 16], F32)
            tmp2 = spool.tile([128, NT, 16], F32)
            nc.vector.tensor_tensor(
                out=tmp[:], in0=s_sb[:], in1=dinva_sb[:].to_broadcast([128, NT, 16]),
                op=ALU.mult,
            )
            nc.vector.tensor_tensor(
                out=tmp2[:], in0=h1_sb[:], in1=dinv2a_sb[:].to_broadcast([128, NT, 16]),
                op=ALU.mult,
            )
            nc.vector.tensor_tensor(out=tmp[:], in0=tmp[:], in1=tmp2[:], op=ALU.add)
            nc.vector.tensor_tensor(out=tmp[:], in0=tmp[:], in1=b1_sb[:], op=ALU.add)
            relu_sb = spool.tile([128, NT, 16], F32)
            nc.scalar.activation(out=relu_sb[:], in_=tmp[:], func=AF.Relu)
            v2_sb = spool.tile([128, NT, 16], BF16)
            nc.vector.tensor_tensor(
                out=v2_sb[:], in0=relu_sb[:],
                in1=dinva_sb[:].to_broadcast([128, NT, 16]), op=ALU.mult,
            )
            nc.sync.dma_start(
                out=relu1.rearrange("(t p) f -> p t f", p=128), in_=relu_sb[:]
            )
            nc.sync.dma_start(out=v2.rearrange("(t p) f -> p t f", p=128), in_=v2_sb[:])
    _legalize_waits(nc)
    return nc


def build_C(NT, nchunks, chunks):
    """s2 -> agg2 -> @W2 + b2 -> log_softmax."""
    nc = bass.Bass()
    g = nc.dram_tensor("g", [128, nchunks, 16], BF16, kind="ExternalInput")
    dstid = nc.dram_tensor("dstid", [128, nchunks], BF16, kind="ExternalInput")
    iota = nc.dram_tensor("iota", [128, SC, GROUP], BF16, kind="ExternalInput")
    relu1 = nc.dram_tensor("relu1", [NT * 128, 16], F32, kind="ExternalInput")
    dinva = nc.dram_tensor("dinva", [128, NT], F32, kind="ExternalInput")
    dinv2a = nc.dram_tensor("dinv2a", [128, NT], F32, kind="ExternalInput")
    b2rep = nc.dram_tensor("b2rep", [128, NT, 16], F32, kind="ExternalInput")
    ident = nc.dram_tensor("ident", [128, 128], F32, kind="ExternalInput")
    W2b = nc.dram_tensor("W2b", [16, 16], BF16, kind="ExternalInput")
    outd = nc.dram_tensor("outd", [NT * 128, 16], F32, kind="ExternalOutput")
    with tile.TileContext(nc) as tc:
        with (
            tc.tile_pool(name="sbuf", bufs=2) as pool,
            tc.tile_pool(name="stat", bufs=1) as spool,
            tc.tile_pool(name="psum", bufs=4, space="PSUM") as pp,
            tc.tile_pool(name="psumt", bufs=2, space="PSUM") as ppt,
        ):
            dstid_sb = spool.tile([128, nchunks], BF16)
            nc.sync.dma_start(out=dstid_sb[:], in_=dstid[:])
            iota_sb = spool.tile([128, SC, GROUP], BF16)
            nc.sync.dma_start(out=iota_sb[:], in_=iota[:])
            r1_sb = spool.tile([128, NT, 16], F32)
            nc.sync.dma_start(
                out=r1_sb[:], in_=relu1.rearrange("(t p) f -> p t f", p=128)
            )
            dinva_sb = spool.tile([128, NT], F32)
            nc.sync.dma_start(out=dinva_sb[:], in_=dinva[:])
            dinv2a_sb = spool.tile([128, NT], F32)
            nc.sync.dma_start(out=dinv2a_sb[:], in_=dinv2a[:])
            b2_sb = spool.tile([128, NT, 16], F32)
            nc.sync.dma_start(out=b2_sb[:], in_=b2rep[:])
            id_sb = spool.tile([128, 128], F32)
            nc.sync.dma_start(out=id_sb[:], in_=ident[:])
            w2_sb = spool.tile([16, 16], BF16)
            nc.sync.dma_start(out=w2_sb[:], in_=W2b[:])
            s_sb = spool.tile([128, NT, 16], F32)

            _emit_segsum(nc, tc, pool, spool, pp, g, dstid_sb, iota_sb, chunks, s_sb, nchunks)

            agg = spool.tile([128, NT, 16], F32)
            tmp2 = spool.tile([128, NT, 16], F32)
            nc.vector.tensor_tensor(
                out=agg[:], in0=s_sb[:], in1=dinva_sb[:].to_broadcast([128, NT, 16]),
                op=ALU.mult,
            )
            nc.vector.tensor_tensor(
                out=tmp2[:], in0=r1_sb[:], in1=dinv2a_sb[:].to_broadcast([128, NT, 16]),
                op=ALU.mult,
            )
            nc.vector.tensor_tensor(out=agg[:], in0=agg[:], in1=tmp2[:], op=ALU.add)

            z_sb = spool.tile([128, NT, 16], F32)
            for t in range(NT):
                trps = ppt.tile([16, 128], F32, tag="trps")
                nc.tensor.transpose(out=trps[:], in_=agg[:, t, :], identity=id_sb[:])
                aggT = pool.tile([16, 128], BF16, tag="aggT")
                nc.scalar.copy(out=aggT[:], in_=trps[:])
                zps = ppt.tile([128, 16], F32, tag="zps")
                nc.tensor.matmul(
                    out=zps[:], lhsT=aggT[:], rhs=w2_sb[:], start=True, stop=True
                )
                nc.scalar.copy(out=z_sb[:, t, :], in_=zps[:])

            nc.vector.tensor_tensor(out=z_sb[:], in0=z_sb[:], in1=b2_sb[:], op=ALU.add)
            m_sb = spool.tile([128, NT], F32)
            nc.vector.tensor_reduce(
                out=m_sb[:], in_=z_sb[:], axis=mybir.AxisListType.X, op=ALU.max
            )
            zc = spool.tile([128, NT, 16], F32)
            nc.vector.tensor_tensor(
                out=zc[:], in0=z_sb[:], in1=m_sb[:].to_broadcast([128, NT, 16]),
                op=ALU.subtract,
            )
            e_sb = spool.tile([128, NT, 16], F32)
            nc.scalar.activation(out=e_sb[:], in_=zc[:], func=AF.Exp)
            ss = spool.tile([128, NT], F32)
            nc.vector.tensor_reduce(
                out=ss[:], in_=e_sb[:], axis=mybir.AxisListType.X, op=ALU.add
            )
            lse = spool.tile([128, NT], F32)
            nc.scalar.activation(out=lse[:], in_=ss[:], func=AF.Ln)
            o_sb = spool.tile([128, NT, 16], F32)
            nc.vector.tensor_tensor(
                out=o_sb[:], in0=zc[:], in1=lse[:].to_broadcast([128, NT, 16]),
                op=ALU.subtract,
            )
            nc.sync.dma_start(out=outd.rearrange("(t p) f -> p t f", p=128), in_=o_sb[:])
    _legalize_waits(nc)
    return nc


# ---------------------------------------------------------------------------
# host side
# ---------------------------------------------------------------------------


def _preprocess(edge_index, n_nodes, per_core):
    """Sort edges by dst, build common-across-cores slot/chunk structure."""
    src = np.asarray(edge_index[0])
    dst = np.asarray(edge_index[1])
    deg = np.bincount(dst, minlength=n_nodes).astype(np.float32) + 1.0
    dinv = 1.0 / np.sqrt(deg)

    order = np.argsort(dst, kind="stable")
    sdst = dst[order]
    ssrc = src[order]

    NT = (per_core + 127) // 128
    padded = NT * 128
    ngroups = padded // GROUP

    bounds = np.searchsorted(sdst, np.arange(N_CORES + 1) * per_core)
    core_grp_cnt = np.zeros((N_CORES, ngroups), np.int64)
    core_edges = []
    for c in range(N_CORES):
        lo, hi = bounds[c], bounds[c + 1]
        ld = sdst[lo:hi] - c * per_core
        grp = ld >> 5
        core_grp_cnt[c] = np.bincount(grp, minlength=ngroups)
        core_edges.append((ld, ssrc[lo:hi]))

    nchunk_g = np.maximum((core_grp_cnt.max(axis=0) + 127) // 128, 1)
    chunk_base = np.concatenate([[0], np.cumsum(nchunk_g)])
    nchunks = int(chunk_base[-1])
    # pad nchunks to a multiple of 4 for tidiness
    chunks = []
    for gi in range(ngroups):
        for k in range(nchunk_g[gi]):
            chunks.append((gi, k == 0, k == nchunk_g[gi] - 1))

    dstid_arrs, sidx_arrs = [], []
    for c in range(N_CORES):
        ld, esrc = core_edges[c]
        grp = ld >> 5
        # rank of each edge within its group (edges sorted by dst => grouped)
        gstart = np.concatenate([[0], np.cumsum(core_grp_cnt[c])])
        rank = np.arange(len(ld)) - np.repeat(gstart[:-1], core_grp_cnt[c])
        slot = chunk_base[grp] * 128 + rank
        nslots = nchunks * 128
        dstid_slots = np.full(nslots, -1.0, np.float32)
        dstid_slots[slot] = (ld & 31).astype(np.float32)
        sidx_slots = np.zeros(nslots, np.int64)
        sidx_slots[slot] = esrc
        dstid_arrs.append(
            dstid_slots.reshape(nchunks, 128).T.astype(NPBF16).copy()
        )
        sidx_arrs.append(sidx_slots.reshape(nchunks, 128).T.copy())
    return dinv, NT, nchunks, chunks, dstid_arrs, sidx_arrs


_CACHE = {}
LAST_HW_NS = None
LAST_TIMES = {}


def _record(tag, res, t_wall):
    """Accumulate per-dispatch HW time (when tracing active) + wall time."""
    global LAST_HW_NS
    LAST_TIMES[tag] = t_wall
    if res.exec_time_ns is not None:
        LAST_HW_NS = (LAST_HW_NS or 0) + res.exec_time_ns


def _kernel_impl(x, W1, b1, W2, b2, edge_index, n_nodes, per_core):
    x = np.asarray(x, dtype=np.float32)
    W1 = np.asarray(W1, dtype=np.float32)
    b1 = np.asarray(b1, dtype=np.float32)
    W2 = np.asarray(W2, dtype=np.float32)
    b2 = np.asarray(b2, dtype=np.float32)
    edge_index = np.asarray(edge_index)
    fin = x.shape[1]

    global LAST_HW_NS
    LAST_HW_NS = None
    LAST_TIMES.clear()
    import time as _time

    t0 = _time.time()
    dinv, NT, nchunks, chunks, dstid_arrs, sidx_arrs = _preprocess(
        edge_index, n_nodes, per_core
    )
    LAST_TIMES["preprocess"] = _time.time() - t0
    padded = NT * 128
    cores = list(range(N_CORES))

    key = (n_nodes, per_core, nchunks)
    if key not in _CACHE:
        _CACHE[key] = (
            build_A(NT, fin),
            build_B(NT, nchunks, chunks),
            build_C(NT, nchunks, chunks),
        )
    ncA, ncB, ncC = _CACHE[key]

    # ---- dispatch A: h1 = x @ W1 ----
    FC = fin // 128
    W1bf = W1.astype(NPBF16)
    W1r = W1bf.reshape(FC, 128, 16).transpose(1, 0, 2).copy()  # [128, FC, 16]
    in_A = []
    for c in cores:
        xs = x[c * per_core : (c + 1) * per_core]
        xp = np.zeros((padded, fin), np.float32)
        xp[: xs.shape[0]] = xs
        xTr = (
            xp.T.astype(NPBF16).reshape(FC, 128, padded).transpose(1, 0, 2).copy()
        )  # [128, FC, padded]
        in_A.append({"xT": xTr, "W1b": W1r})
    t0 = _time.time()
    resA = run_bass_kernel_spmd(ncA, in_A, core_ids=cores)
    _record("dispatchA", resA, _time.time() - t0)
    h1s = [resA.results[c]["h1"] for c in cores]  # [padded, 16] f32

    # ---- host gather for layer 1 ----
    u1 = np.concatenate([h1s[c][:per_core] for c in cores], axis=0)
    u1 *= dinv[:, None]

    # static scale/bias arrays per core
    iota_np = np.broadcast_to(
        np.arange(GROUP, dtype=np.float32), (128, SC, GROUP)
    ).astype(NPBF16)
    ident_np = np.eye(128, dtype=np.float32)
    W2bf = W2.astype(NPBF16)
    dinva_c, dinv2a_c, b1rep, b2rep = [], [], None, None
    for c in cores:
        dv = np.ones(padded, np.float32)
        dv[:per_core] = dinv[c * per_core : (c + 1) * per_core]
        dinva_c.append(dv.reshape(NT, 128).T.copy())
        dinv2a_c.append((dv * dv).reshape(NT, 128).T.copy())
    b1rep = np.broadcast_to(b1, (128, NT, 16)).astype(np.float32).copy()
    b2rep = np.broadcast_to(b2, (128, NT, 16)).astype(np.float32).copy()

    def gath(table, c):
        return table[sidx_arrs[c]].astype(NPBF16)  # [128, nchunks, 16]

    # ---- dispatch B ----
    in_B = []
    for c in cores:
        in_B.append(
            {
                "g": gath(u1, c),
                "dstid": dstid_arrs[c],
                "iota": iota_np,
                "h1": h1s[c],
                "dinva": dinva_c[c],
                "dinv2a": dinv2a_c[c],
                "b1rep": b1rep,
            }
        )
    t0 = _time.time()
    resB = run_bass_kernel_spmd(ncB, in_B, core_ids=cores)
    _record("dispatchB", resB, _time.time() - t0)
    relu1s = [resB.results[c]["relu1"] for c in cores]
    v2s = [resB.results[c]["v2"] for c in cores]

    # ---- host gather for layer 2 ----
    v2full = np.concatenate(
        [v2s[c][:per_core].astype(np.float32) for c in cores], axis=0
    )

    # ---- dispatch C ----
    in_C = []
    for c in cores:
        in_C.append(
            {
                "g": gath(v2full, c),
                "dstid": dstid_arrs[c],
                "iota": iota_np,
                "relu1": relu1s[c],
                "dinva": dinva_c[c],
                "dinv2a": dinv2a_c[c],
                "b2rep": b2rep,
                "ident": ident_np,
                "W2b": W2bf,
            }
        )
    t0 = _time.time()
    resC = run_bass_kernel_spmd(ncC, in_C, core_ids=cores)
    _record("dispatchC", resC, _time.time() - t0)
    out = np.concatenate(
        [resC.results[c]["outd"][:per_core] for c in cores], axis=0
    ).astype(np.float32)
    return out


def kernel(x, W1, b1, W2, b2, edge_index):
    return _kernel_impl(x, W1, b1, W2, b2, edge_index, 100000, 12500)



# revision 32
# speedup vs baseline: 2.5553x; 2.5553x over previous
"""GCN (2-layer, PyG GCNConv semantics) on 8 Trainium2 NeuronCores.

Strategy (dst-shard, graph-parallel), v2:
- Nodes sharded contiguously across 8 cores (12500 dsts/core).
- 3 SPMD dispatches:
    A: u1 = dinv * (x @ W1)            (x pre-transposed bf16, 4KB DMA runs)
    B: s1 = segsum(g1); agg1 = dinv*(s1+u1own)+b1; r1 = relu;
       v2 = dinv*r1; t2 = v2 @ W2      (outputs only t2, 0.4MB)
    C: s2 = segsum(g2); z = dinv*(s2+t2own)+b2; out = log_softmax(z)
- Segment-sum: edges packed 8-per-slot by destination; per 128-dst
  block, the first T_ID=4 slots of every dst go to "identity" chunks
  (slot partition == dst row, lhsT = static identity - no one-hot
  work), remaining slots to ~1 "overflow" chunk routed by an is_equal
  one-hot. All chunks of a block accumulate into one PSUM tile
  [128, 16f, 8sub]; one DVE reduce per block sums the 8 subslots.
  Chunk structure is common across cores (max-over-cores sizing) so a
  single SPMD program serves all 8 cores.
- The two per-edge value gathers (u1[src]/t2[src] for 3.2M edges) run
  on the host between dispatches (every on-device gather primitive in
  this toolchain was measured unusable: indirect DMA ~1.6us/row,
  GPSIMD gather ucode unloadable under this walrus build).
"""
import os
import sys
import numpy as np

sys.path.insert(0, "/opt/trn_rl_repo")

try:
    # NTFF profiling glue: the image's antenv lacks axon_hooks, which makes
    # run_bass_kernel_spmd(trace=True) crash. Provide it (and register the
    # ctypes hook) so tracing works when BASS_TRACE is set; harmless if not.
    import types as _types

    if "antenv.axon_hooks" not in sys.modules:
        _m = _types.ModuleType("antenv.axon_hooks")
        _st = {}
        _m.set_axon_ntff_profile_hook = lambda h: _st.__setitem__("h", h)
        _m.get_axon_ntff_profile_hook = lambda: _st.get("h")
        sys.modules["antenv.axon_hooks"] = _m
        from trn_agent_boot.trn_boot import _ntff_profile_via_ctypes

        _m.set_axon_ntff_profile_hook(
            _ntff_profile_via_ctypes("/opt/axon/libaxon_pjrt.so")
        )
except Exception:
    pass

import ml_dtypes
import concourse.bass as bass
import concourse.mybir as mybir
import concourse.tile as tile
from concourse.vector_clock import ScopedClock
import concourse.bass_utils as _bu
from concourse.bass_utils import run_bass_kernel_spmd

_orig_upload = _bu.upload_artifacts


def _safe_upload(tmpdir):
    try:
        return _orig_upload(tmpdir)
    except Exception:
        return "local://" + tmpdir


_bu.upload_artifacts = _safe_upload

BF16 = mybir.dt.bfloat16
F32 = mybir.dt.float32
AF = mybir.ActivationFunctionType
ALU = mybir.AluOpType
NPBF16 = ml_dtypes.bfloat16

N_CORES = 8
PER_CORE = 12500
NT = 98              # 128-dst tiles per core (12544 padded)
PADDED = NT * 128
PACK = 8             # edges per slot (matmul N = 16 feats x PACK)
T_ID = 4             # identity chunks per block (slots 0..3 of each dst)
SC = 16              # chunks per g superchunk (DMA batch)
SCOV = 16            # overflow chunks per is_equal batch

# ---------------------------------------------------------------------------
# walrus workaround: only ONE sync-wait command per instruction is accepted.
# ---------------------------------------------------------------------------


def _patched_drain_and_barrier(self, tick_clock, wait_clock):
    nc = self.nc
    carrier = nc.sync.nop(nofuse=True, hint="drain_wait_carrier")
    wait_clock.add_sem_waits(carrier.ins, ScopedClock({None: tick_clock.global_clock}))
    si = carrier.ins.sync_info
    waits = list(si.on_wait or []) if si else []
    if len(waits) > 1:
        si.on_wait = waits[:1]
        for i in range(1, len(waits)):
            extra = nc.sync.nop(nofuse=True, hint="drain_wait_carrier")
            extra.ins.sync_info = mybir.SyncInfo(on_wait=waits[i : i + 1], on_update=[])
    nc.sync.drain()
    nc.all_engine_barrier()
    assert self.sems is not None
    popped = nc._tile_sem_poison_stack.pop()
    assert popped is self._sem_poison
    nc.clear_and_free_semaphores(list(self.sems.allocated().values()))
    nc.all_engine_barrier()


tile.TileContext._drain_and_barrier = _patched_drain_and_barrier


def _legalize_waits(nc, max_waits=1):
    n = [0]

    def mk_nop(engine, waits):
        n[0] += 1
        return mybir.InstNoOp(
            name=f"waitnop-{n[0]}",
            engine=engine,
            ins=[],
            outs=[],
            sync_info=mybir.SyncInfo(on_wait=list(waits), on_update=[]),
            text_hint="wait_carrier",
        )

    for f in nc.m.functions:
        for bb in f.blocks:
            out = []
            changed = False
            for inst in bb.instructions:
                si = inst.sync_info
                waits = list(si.on_wait or []) if si else []
                if len(waits) > max_waits:
                    changed = True
                    for i in range(0, len(waits) - max_waits, max_waits):
                        out.append(mk_nop(inst.engine, waits[i : i + max_waits]))
                    si.on_wait = waits[len(waits) - max_waits :]
                out.append(inst)
            if changed:
                bb.instructions = out
    return nc


# ---------------------------------------------------------------------------
# device kernel builders
# ---------------------------------------------------------------------------


def build_A(FC=4):
    """u1 = dinv * (x @ W1). xT host layout [128, NT, FC, 128] bf16."""
    nc = bass.Bass()
    xT = nc.dram_tensor("xT", [128, NT, FC, 128], BF16, kind="ExternalInput")
    W1b = nc.dram_tensor("W1b", [128, FC, 16], BF16, kind="ExternalInput")
    dinva = nc.dram_tensor("dinva", [128, NT], F32, kind="ExternalInput")
    u1 = nc.dram_tensor("u1", [128, NT, 16], BF16, kind="ExternalOutput")
    TB = 4  # node-tiles per DMA batch (4KB per partition)
    with tile.TileContext(nc) as tc:
        with (
            tc.tile_pool(name="sbuf", bufs=3) as pool,
            tc.tile_pool(name="stat", bufs=1) as spool,
            tc.tile_pool(name="psum", bufs=8, space="PSUM") as pp,
        ):
            w1 = spool.tile([128, FC, 16], BF16)
            nc.sync.dma_start(out=w1[:], in_=W1b[:])
            da = spool.tile([128, NT], F32)
            nc.sync.dma_start(out=da[:], in_=dinva[:])
            u1_sb = spool.tile([128, NT, 16], BF16)
            for t0 in range(0, NT, TB):
                tb = min(TB, NT - t0)
                xt = pool.tile([128, TB, FC, 128], BF16, tag="xt")
                nc.sync.dma_start(out=xt[:, :tb], in_=xT[:, t0 : t0 + tb])
                for i in range(tb):
                    ps = pp.tile([128, 16], F32, tag="hps")
                    for fc in range(FC):
                        nc.tensor.matmul(
                            out=ps[:],
                            lhsT=xt[:, i, fc, :],
                            rhs=w1[:, fc, :],
                            start=(fc == 0),
                            stop=(fc == FC - 1),
                        )
                    t = t0 + i
                    nc.vector.tensor_tensor(
                        out=u1_sb[:, t, :],
                        in0=ps[:],
                        in1=da[:, t : t + 1].to_broadcast([128, 16]),
                        op=ALU.mult,
                    )
            nc.sync.dma_start(out=u1[:], in_=u1_sb[:])
    return _legalize_waits(nc)


def _emit_segsum(
    nc, pool, pp, g, dstid_ov_sb, iota_sb, id_sb, blocks_nov, CH, CHOV, on_stripe
):
    """Per-block psum scatter + subslot reduce, delivered in 4-block stripes.

    Per block: T_ID identity chunks (lhsT = id_sb) + blocks_nov[b] overflow
    chunks (lhsT = is_equal one-hot from dstid_ov). All chunks of a block
    accumulate into one PSUM sub-tile; 4 blocks share a bank. After each
    stripe's DVE reduce, on_stripe(b0, nb, s4) consumes the [128, nb, 16]
    f32 stripe so the epilogue overlaps the remaining scatter."""
    g_sc = None
    s_ov = None
    P4 = None
    q = 0
    jov = 0
    NB = len(blocks_nov)
    for b, nov in enumerate(blocks_nov):
        if b % 4 == 0:
            P4 = pp.tile([128, 4, 16, PACK], F32, tag="pblk")
        nch = T_ID + nov
        for k in range(nch):
            if q % SC == 0:
                wsc = min(SC, CH - q)
                g_sc = pool.tile([128, SC, 16, PACK], BF16, tag="gsc")
                eng = nc.sync if (q // SC) % 2 == 0 else nc.scalar
                eng.dma_start(out=g_sc[:, :wsc], in_=g[:, q : q + wsc])
            if k >= T_ID:
                if jov % SCOV == 0:
                    wov = min(SCOV, CHOV - jov)
                    s_ov = pool.tile([128, SCOV, 128], BF16, tag="sov")
                    nc.vector.tensor_tensor(
                        out=s_ov[:, :wov, :],
                        in0=dstid_ov_sb[:, jov : jov + wov].to_broadcast(
                            [128, wov, 128]
                        ),
                        in1=iota_sb[:, :wov, :],
                        op=ALU.is_equal,
                    )
                lhsT = s_ov[:, jov % SCOV, :]
                jov += 1
            else:
                lhsT = id_sb[:]
            nc.tensor.matmul(
                out=P4[:, b % 4],
                lhsT=lhsT,
                rhs=g_sc[:, q % SC],
                start=(k == 0),
                stop=(k == nch - 1),
            )
            q += 1
        if b % 4 == 3 or b == NB - 1:
            b0 = (b // 4) * 4
            nb = b - b0 + 1
            s4 = pool.tile([128, 4, 16], F32, tag="s4")
            nc.vector.tensor_reduce(
                out=s4[:, :nb],
                in_=P4[:, :nb],
                axis=mybir.AxisListType.X,
                op=ALU.add,
            )
            on_stripe(b0, nb, s4)


def build_B(CH, CHOV, blocks_nov):
    """s1 -> agg1 -> relu -> v2 -> t2 = v2 @ W2 (sole output), striped."""
    nc = bass.Bass()
    g = nc.dram_tensor("g", [128, CH, 16, PACK], BF16, kind="ExternalInput")
    dstid = nc.dram_tensor("dstid", [128, CHOV], BF16, kind="ExternalInput")
    iota = nc.dram_tensor("iota", [128, SCOV, 128], BF16, kind="ExternalInput")
    u1own = nc.dram_tensor("u1own", [128, NT, 16], BF16, kind="ExternalInput")
    dinva = nc.dram_tensor("dinva", [128, NT], F32, kind="ExternalInput")
    b1r = nc.dram_tensor("b1r", [128, 1, 16], F32, kind="ExternalInput")
    W2q = nc.dram_tensor("W2q", [64, 4, 16], BF16, kind="ExternalInput")
    identT = nc.dram_tensor("identT", [128, 128], BF16, kind="ExternalInput")
    t2 = nc.dram_tensor("t2", [128, NT, 16], BF16, kind="ExternalOutput")
    with tile.TileContext(nc) as tc:
        with (
            tc.tile_pool(name="sbuf", bufs=3) as pool,
            tc.tile_pool(name="stat", bufs=1) as spool,
            tc.tile_pool(name="psum", bufs=4, space="PSUM") as pp,
            tc.tile_pool(name="psumt", bufs=2, space="PSUM") as ppt,
        ):
            dstid_sb = spool.tile([128, CHOV], BF16)
            nc.sync.dma_start(out=dstid_sb[:], in_=dstid[:])
            iota_sb = spool.tile([128, SCOV, 128], BF16)
            nc.sync.dma_start(out=iota_sb[:], in_=iota[:])
            u1o_bf = spool.tile([128, NT, 16], BF16)
            nc.sync.dma_start(out=u1o_bf[:], in_=u1own[:])
            da = spool.tile([128, NT], F32)
            nc.sync.dma_start(out=da[:], in_=dinva[:])
            b1_sb = spool.tile([128, 1, 16], F32)
            nc.sync.dma_start(out=b1_sb[:], in_=b1r[:])
            w2q_sb = spool.tile([64, 4, 16], BF16)
            nc.sync.dma_start(out=w2q_sb[:], in_=W2q[:])
            id_sb = spool.tile([128, 128], BF16)
            nc.sync.dma_start(out=id_sb[:], in_=identT[:])
            u1o = spool.tile([128, NT, 16], F32)
            nc.scalar.copy(out=u1o[:], in_=u1o_bf[:])
            t2_sb = spool.tile([128, NT, 16], BF16)

            def on_stripe(b0, nb, s4):
                sl = slice(b0, b0 + nb)
                agg = pool.tile([128, 4, 16], F32, tag="agg")
                nc.gpsimd.tensor_tensor(
                    out=agg[:, :nb], in0=s4[:, :nb], in1=u1o[:, sl], op=ALU.add
                )
                nc.gpsimd.tensor_tensor(
                    out=agg[:, :nb], in0=agg[:, :nb],
                    in1=da[:, sl].to_broadcast([128, nb, 16]), op=ALU.mult,
                )
                nc.gpsimd.tensor_tensor(
                    out=agg[:, :nb], in0=agg[:, :nb],
                    in1=b1_sb[:].to_broadcast([128, nb, 16]), op=ALU.add,
                )
                r4 = pool.tile([128, 4, 16], F32, tag="r4")
                nc.scalar.activation(out=r4[:, :nb], in_=agg[:, :nb], func=AF.Relu)
                v4 = pool.tile([128, 4, 16], BF16, tag="v4")
                nc.vector.tensor_tensor(
                    out=v4[:, :nb], in0=r4[:, :nb],
                    in1=da[:, sl].to_broadcast([128, nb, 16]), op=ALU.mult,
                )
                if nb < 4:
                    nc.vector.memset(v4[:, nb:, :], 0.0)
                trps = ppt.tile([64, 128], BF16, tag="trps")
                nc.tensor.transpose(out=trps[:], in_=v4[:], identity=id_sb[:])
                v2T = pool.tile([64, 128], BF16, tag="v2T")
                nc.scalar.copy(out=v2T[:], in_=trps[:])
                z4 = ppt.tile([128, 4, 16], F32, tag="z4")
                for j in range(nb):
                    nc.tensor.matmul(
                        out=z4[:, j], lhsT=v2T[:], rhs=w2q_sb[:, j, :],
                        start=True, stop=True,
                    )
                nc.scalar.copy(out=t2_sb[:, sl, :], in_=z4[:, :nb])

            _emit_segsum(
                nc, pool, pp, g, dstid_sb, iota_sb, id_sb, blocks_nov, CH, CHOV,
                on_stripe,
            )
            nc.sync.dma_start(out=t2[:], in_=t2_sb[:])
    return _legalize_waits(nc)


def build_C(CH, CHOV, blocks_nov):
    """s2 -> z = dinv*(s2 + t2own) + b2 -> log_softmax, striped."""
    nc = bass.Bass()
    g = nc.dram_tensor("g", [128, CH, 16, PACK], BF16, kind="ExternalInput")
    dstid = nc.dram_tensor("dstid", [128, CHOV], BF16, kind="ExternalInput")
    iota = nc.dram_tensor("iota", [128, SCOV, 128], BF16, kind="ExternalInput")
    t2own = nc.dram_tensor("t2own", [128, NT, 16], BF16, kind="ExternalInput")
    dinva = nc.dram_tensor("dinva", [128, NT], F32, kind="ExternalInput")
    b2r = nc.dram_tensor("b2r", [128, 1, 16], F32, kind="ExternalInput")
    identT = nc.dram_tensor("identT", [128, 128], BF16, kind="ExternalInput")
    outd = nc.dram_tensor("outd", [128, NT, 16], F32, kind="ExternalOutput")
    with tile.TileContext(nc) as tc:
        with (
            tc.tile_pool(name="sbuf", bufs=3) as pool,
            tc.tile_pool(name="stat", bufs=1) as spool,
            tc.tile_pool(name="psum", bufs=6, space="PSUM") as pp,
        ):
            dstid_sb = spool.tile([128, CHOV], BF16)
            nc.sync.dma_start(out=dstid_sb[:], in_=dstid[:])
            iota_sb = spool.tile([128, SCOV, 128], BF16)
            nc.sync.dma_start(out=iota_sb[:], in_=iota[:])
            t2o_bf = spool.tile([128, NT, 16], BF16)
            nc.sync.dma_start(out=t2o_bf[:], in_=t2own[:])
            da = spool.tile([128, NT], F32)
            nc.sync.dma_start(out=da[:], in_=dinva[:])
            b2_sb = spool.tile([128, 1, 16], F32)
            nc.sync.dma_start(out=b2_sb[:], in_=b2r[:])
            id_sb = spool.tile([128, 128], BF16)
            nc.sync.dma_start(out=id_sb[:], in_=identT[:])
            t2o = spool.tile([128, NT, 16], F32)
            nc.scalar.copy(out=t2o[:], in_=t2o_bf[:])
            o_sb = spool.tile([128, NT, 16], F32)

            def on_stripe(b0, nb, s4):
                sl = slice(b0, b0 + nb)
                z = pool.tile([128, 4, 16], F32, tag="zs")
                nc.gpsimd.tensor_tensor(
                    out=z[:, :nb], in0=s4[:, :nb], in1=t2o[:, sl], op=ALU.add
                )
                nc.gpsimd.tensor_tensor(
                    out=z[:, :nb], in0=z[:, :nb],
                    in1=da[:, sl].to_broadcast([128, nb, 16]), op=ALU.mult,
                )
                nc.gpsimd.tensor_tensor(
                    out=z[:, :nb], in0=z[:, :nb],
                    in1=b2_sb[:].to_broadcast([128, nb, 16]), op=ALU.add,
                )
                m4 = pool.tile([128, 4], F32, tag="m4")
                nc.vector.tensor_reduce(
                    out=m4[:, :nb], in_=z[:, :nb], axis=mybir.AxisListType.X,
                    op=ALU.max,
                )
                zc = pool.tile([128, 4, 16], F32, tag="zc")
                nc.vector.tensor_tensor(
                    out=zc[:, :nb], in0=z[:, :nb],
                    in1=m4[:, :nb].to_broadcast([128, nb, 16]), op=ALU.subtract,
                )
                e4 = pool.tile([128, 4, 16], F32, tag="e4")
                nc.scalar.activation(out=e4[:, :nb], in_=zc[:, :nb], func=AF.Exp)
                ss = pool.tile([128, 4], F32, tag="ss")
                nc.vector.tensor_reduce(
                    out=ss[:, :nb], in_=e4[:, :nb], axis=mybir.AxisListType.X,
                    op=ALU.add,
                )
                lse = pool.tile([128, 4], F32, tag="lse")
                nc.scalar.activation(out=lse[:, :nb], in_=ss[:, :nb], func=AF.Ln)
                nc.vector.tensor_tensor(
                    out=o_sb[:, sl, :], in0=zc[:, :nb],
                    in1=lse[:, :nb].to_broadcast([128, nb, 16]), op=ALU.subtract,
                )

            _emit_segsum(
                nc, pool, pp, g, dstid_sb, iota_sb, id_sb, blocks_nov, CH, CHOV,
                on_stripe,
            )
            nc.sync.dma_start(out=outd[:], in_=o_sb[:])
    return _legalize_waits(nc)


# ---------------------------------------------------------------------------
# host side
# ---------------------------------------------------------------------------


def _preprocess(edge_index, n_nodes):
    """Sort edges by dst; build the common chunk structure (T_ID identity +
    n_ov overflow chunks per 128-dst block) + per-core slot metadata."""
    src = np.asarray(edge_index[0])
    dst = np.asarray(edge_index[1])
    deg = np.bincount(dst, minlength=n_nodes).astype(np.float32) + 1.0
    dinv = (1.0 / np.sqrt(deg)).astype(np.float32)

    order = np.argsort(dst, kind="stable")
    sdst = dst[order]
    ssrc = src[order]
    bounds = np.searchsorted(sdst, np.arange(N_CORES + 1) * PER_CORE)

    # per-core local in-degree and slot counts
    deg_loc = np.zeros((N_CORES, PADDED), np.int64)
    core_edges = []
    for c in range(N_CORES):
        lo, hi = bounds[c], bounds[c + 1]
        ld = sdst[lo:hi] - c * PER_CORE
        deg_loc[c, : PER_CORE] = np.bincount(ld, minlength=PER_CORE)
        core_edges.append((ld, ssrc[lo:hi]))
    nslots = -(-deg_loc // PACK)                 # [8, PADDED] ceil div
    ovslots = np.maximum(nslots - T_ID, 0)       # [8, PADDED]

    # common structure: overflow chunk count per block = max over cores
    ov_per_block = ovslots.reshape(N_CORES, NT, 128).sum(axis=2)  # [8, NT]
    n_ov = -(-ov_per_block.max(axis=0) // 128)   # [NT]
    blocks_nov = tuple(int(v) for v in n_ov)
    chunk_base = np.concatenate([[0], np.cumsum(T_ID + n_ov)])    # [NT+1]
    CH = int(chunk_base[-1])
    ov_idx_base = np.concatenate([[0], np.cumsum(n_ov)])          # [NT+1]
    CHOV = max(int(ov_idx_base[-1]), 1)

    sent = N_CORES * PADDED  # sentinel row (zeros) in gather tables
    dstid_arrs, sidx_arrs = [], []
    blk_of_dst = np.arange(PADDED) >> 7
    for c in range(N_CORES):
        ov = ovslots[c]
        # exclusive cumsum of overflow slots within each block
        ovc = np.cumsum(ov) - ov
        blk_start = blk_of_dst << 7
        ovbase = ovc - ovc[blk_start]            # [PADDED]
        ld, esrc = core_edges[c]
        gstart = np.concatenate([[0], np.cumsum(deg_loc[c])])
        rank = np.arange(len(ld)) - gstart[ld]
        k_e = rank // PACK
        c_e = rank % PACK
        blk = ld >> 7
        is_id = k_e < T_ID
        q_id = chunk_base[blk] + k_e
        p_id = ld & 127
        ovpos = ovbase[ld] + (k_e - T_ID)
        q_ov = chunk_base[blk] + T_ID + ovpos // 128
        p_ov = ovpos % 128
        q_e = np.where(is_id, q_id, q_ov)
        p_e = np.where(is_id, p_id, p_ov)
        # gather row index: src node -> (core, p, t) -> core*PADDED + p*NT + t
        sc_, rr = esrc // PER_CORE, esrc % PER_CORE
        grow = sc_ * PADDED + (rr % 128) * NT + rr // 128
        sidx = np.full((128, CH, PACK), sent, np.int64)
        sidx[p_e, q_e, c_e] = grow
        dstid = np.full((128, CHOV), -1.0, np.float32)
        m = (~is_id) & (c_e == 0)
        qovc = ov_idx_base[blk[m]] + ovpos[m] // 128
        dstid[p_ov[m], qovc] = (ld[m] & 127).astype(np.float32)
        dstid_arrs.append(dstid.astype(NPBF16))
        sidx_arrs.append(sidx)
    return dinv, CH, CHOV, blocks_nov, dstid_arrs, sidx_arrs


_CACHE = {}
LAST_HW_NS = None
LAST_TIMES = {}


def _record(tag, res, t_wall):
    global LAST_HW_NS
    LAST_TIMES[tag] = t_wall
    if res.exec_time_ns is not None:
        LAST_HW_NS = (LAST_HW_NS or 0) + res.exec_time_ns


def _gather_g(table, sidx):
    """table [8*PADDED+1, 16] bf16, sidx [128, CH, PACK] -> [128, CH, 16, PACK]."""
    vals = table[sidx]  # [128, CH, PACK, 16]
    return np.ascontiguousarray(vals.transpose(0, 1, 3, 2))


def kernel(x, W1, b1, W2, b2, edge_index):
    global LAST_HW_NS
    LAST_HW_NS = None
    LAST_TIMES.clear()
    import time as _time

    x = np.asarray(x, dtype=np.float32)
    W1 = np.asarray(W1, dtype=np.float32)
    b1 = np.asarray(b1, dtype=np.float32)
    W2 = np.asarray(W2, dtype=np.float32)
    b2 = np.asarray(b2, dtype=np.float32)
    edge_index = np.asarray(edge_index)
    n_nodes, fin = x.shape
    FC = fin // 128

    t0 = _time.time()
    dinv, CH, CHOV, blocks_nov, dstid_arrs, sidx_arrs = _preprocess(
        edge_index, n_nodes
    )
    LAST_TIMES["preprocess"] = _time.time() - t0

    key = (n_nodes, CH, CHOV, blocks_nov)
    if key not in _CACHE:
        _CACHE[key] = (
            build_A(FC),
            build_B(CH, CHOV, blocks_nov),
            build_C(CH, CHOV, blocks_nov),
        )
    ncA, ncB, ncC = _CACHE[key]
    cores = list(range(N_CORES))

    # ---- static per-core arrays ----
    t0 = _time.time()
    W1r = np.ascontiguousarray(
        W1.astype(NPBF16).reshape(FC, 128, 16).transpose(1, 0, 2)
    )
    dinva_c = []
    for c in cores:
        dv = np.ones(PADDED, np.float32)
        dv[:PER_CORE] = dinv[c * PER_CORE : (c + 1) * PER_CORE]
        dinva_c.append(np.ascontiguousarray(dv.reshape(NT, 128).T))
    iota_np = np.ascontiguousarray(
        np.broadcast_to(np.arange(128, dtype=np.float32), (128, SCOV, 128))
    ).astype(NPBF16)
    b1r = np.ascontiguousarray(np.broadcast_to(b1, (128, 1, 16)).astype(np.float32))
    b2r = np.ascontiguousarray(np.broadcast_to(b2, (128, 1, 16)).astype(np.float32))
    W2bf = W2.astype(NPBF16)
    w2q = np.zeros((64, 4, 16), NPBF16)
    for j in range(4):
        w2q[16 * j : 16 * j + 16, j] = W2bf
    ident_np = np.eye(128, dtype=np.float32).astype(NPBF16)

    # ---- dispatch A ----
    in_A = []
    for c in cores:
        xs = x[c * PER_CORE : (c + 1) * PER_CORE]
        xp = np.zeros((PADDED, fin), NPBF16)
        xp[: xs.shape[0]] = xs.astype(NPBF16)
        xTr = np.ascontiguousarray(
            xp.reshape(NT, 128, FC, 128).transpose(3, 0, 2, 1)
        )  # [128 f_lo, NT, FC, 128 n]
        in_A.append({"xT": xTr, "W1b": W1r, "dinva": dinva_c[c]})
    LAST_TIMES["prepA"] = _time.time() - t0
    t0 = _time.time()
    resA = run_bass_kernel_spmd(ncA, in_A, core_ids=cores)
    _record("dispatchA", resA, _time.time() - t0)
    u1s = [resA.results[c]["u1"] for c in cores]  # [128, NT, 16] bf16

    # ---- host gather for layer 1 ----
    t0 = _time.time()
    table1 = np.concatenate(
        [u1s[c].reshape(PADDED, 16) for c in cores] + [np.zeros((1, 16), NPBF16)],
        axis=0,
    )
    in_B = []
    for c in cores:
        in_B.append(
            {
                "g": _gather_g(table1, sidx_arrs[c]),
                "dstid": dstid_arrs[c],
                "iota": iota_np,
                "u1own": u1s[c],
                "dinva": dinva_c[c],
                "b1r": b1r,
                "W2q": w2q,
                "identT": ident_np,
            }
        )
    LAST_TIMES["gather1"] = _time.time() - t0
    t0 = _time.time()
    resB = run_bass_kernel_spmd(ncB, in_B, core_ids=cores)
    _record("dispatchB", resB, _time.time() - t0)
    t2s = [resB.results[c]["t2"] for c in cores]

    # ---- host gather for layer 2 ----
    t0 = _time.time()
    table2 = np.concatenate(
        [t2s[c].reshape(PADDED, 16) for c in cores] + [np.zeros((1, 16), NPBF16)],
        axis=0,
    )
    in_C = []
    for c in cores:
        in_C.append(
            {
                "g": _gather_g(table2, sidx_arrs[c]),
                "dstid": dstid_arrs[c],
                "iota": iota_np,
                "t2own": t2s[c],
                "dinva": dinva_c[c],
                "b2r": b2r,
                "identT": ident_np,
            }
        )
    LAST_TIMES["gather2"] = _time.time() - t0
    t0 = _time.time()
    resC = run_bass_kernel_spmd(ncC, in_C, core_ids=cores)
    _record("dispatchC", resC, _time.time() - t0)
    out = np.concatenate(
        [
            resC.results[c]["outd"].transpose(1, 0, 2).reshape(PADDED, 16)[:PER_CORE]
            for c in cores
        ],
        axis=0,
    ).astype(np.float32)
    return out


# revision 35
# speedup vs baseline: 3.0151x; 1.1799x over previous
"""GCN (2-layer, PyG GCNConv semantics) on 8 Trainium2 NeuronCores.

Strategy (dst-shard, graph-parallel), v2:
- Nodes sharded contiguously across 8 cores (12500 dsts/core).
- 3 SPMD dispatches:
    A: u1 = dinv * (x @ W1)            (x pre-transposed bf16, 4KB DMA runs)
    B: s1 = segsum(g1); agg1 = dinv*(s1+u1own)+b1; r1 = relu;
       v2 = dinv*r1; t2 = v2 @ W2      (outputs only t2, 0.4MB)
    C: s2 = segsum(g2); z = dinv*(s2+t2own)+b2; out = log_softmax(z)
- Segment-sum: edges packed 8-per-slot by destination; per 128-dst
  block, the first T_ID=4 slots of every dst go to "identity" chunks
  (slot partition == dst row, lhsT = static identity - no one-hot
  work), remaining slots to ~1 "overflow" chunk routed by an is_equal
  one-hot. All chunks of a block accumulate into one PSUM tile
  [128, 16f, 8sub]; one DVE reduce per block sums the 8 subslots.
  Chunk structure is common across cores (max-over-cores sizing) so a
  single SPMD program serves all 8 cores.
- The two per-edge value gathers (u1[src]/t2[src] for 3.2M edges) run
  on the host between dispatches (every on-device gather primitive in
  this toolchain was measured unusable: indirect DMA ~1.6us/row,
  GPSIMD gather ucode unloadable under this walrus build).
"""
import os
import sys
import numpy as np

sys.path.insert(0, "/opt/trn_rl_repo")

try:
    # NTFF profiling glue: the image's antenv lacks axon_hooks, which makes
    # run_bass_kernel_spmd(trace=True) crash. Provide it (and register the
    # ctypes hook) so tracing works when BASS_TRACE is set; harmless if not.
    import types as _types

    if "antenv.axon_hooks" not in sys.modules:
        _m = _types.ModuleType("antenv.axon_hooks")
        _st = {}
        _m.set_axon_ntff_profile_hook = lambda h: _st.__setitem__("h", h)
        _m.get_axon_ntff_profile_hook = lambda: _st.get("h")
        sys.modules["antenv.axon_hooks"] = _m
        from trn_agent_boot.trn_boot import _ntff_profile_via_ctypes

        _m.set_axon_ntff_profile_hook(
            _ntff_profile_via_ctypes("/opt/axon/libaxon_pjrt.so")
        )
except Exception:
    pass

import ml_dtypes
import concourse.bass as bass
import concourse.mybir as mybir
import concourse.tile as tile
from concourse.vector_clock import ScopedClock
import concourse.bass_utils as _bu
from concourse.bass_utils import run_bass_kernel_spmd

_orig_upload = _bu.upload_artifacts


def _safe_upload(tmpdir):
    try:
        return _orig_upload(tmpdir)
    except Exception:
        return "local://" + tmpdir


_bu.upload_artifacts = _safe_upload

BF16 = mybir.dt.bfloat16
F32 = mybir.dt.float32
AF = mybir.ActivationFunctionType
ALU = mybir.AluOpType
NPBF16 = ml_dtypes.bfloat16

N_CORES = 8
PER_CORE = 12500
NT = 98              # 128-dst tiles per core (12544 padded)
PADDED = NT * 128
PACK = 8             # edges per slot (matmul N = 16 feats x PACK)
T_ID = 4             # identity chunks per block (slots 0..3 of each dst)
SC = 32              # chunks per g superchunk (DMA batch)
SCOV = 16            # overflow chunks per is_equal batch

# ---------------------------------------------------------------------------
# walrus workaround: only ONE sync-wait command per instruction is accepted.
# ---------------------------------------------------------------------------


def _patched_drain_and_barrier(self, tick_clock, wait_clock):
    nc = self.nc
    carrier = nc.sync.nop(nofuse=True, hint="drain_wait_carrier")
    wait_clock.add_sem_waits(carrier.ins, ScopedClock({None: tick_clock.global_clock}))
    si = carrier.ins.sync_info
    waits = list(si.on_wait or []) if si else []
    if len(waits) > 1:
        si.on_wait = waits[:1]
        for i in range(1, len(waits)):
            extra = nc.sync.nop(nofuse=True, hint="drain_wait_carrier")
            extra.ins.sync_info = mybir.SyncInfo(on_wait=waits[i : i + 1], on_update=[])
    nc.sync.drain()
    nc.all_engine_barrier()
    assert self.sems is not None
    popped = nc._tile_sem_poison_stack.pop()
    assert popped is self._sem_poison
    nc.clear_and_free_semaphores(list(self.sems.allocated().values()))
    nc.all_engine_barrier()


tile.TileContext._drain_and_barrier = _patched_drain_and_barrier


def _legalize_waits(nc, max_waits=1):
    n = [0]

    def mk_nop(engine, waits):
        n[0] += 1
        return mybir.InstNoOp(
            name=f"waitnop-{n[0]}",
            engine=engine,
            ins=[],
            outs=[],
            sync_info=mybir.SyncInfo(on_wait=list(waits), on_update=[]),
            text_hint="wait_carrier",
        )

    for f in nc.m.functions:
        for bb in f.blocks:
            out = []
            changed = False
            for inst in bb.instructions:
                si = inst.sync_info
                waits = list(si.on_wait or []) if si else []
                if len(waits) > max_waits:
                    changed = True
                    for i in range(0, len(waits) - max_waits, max_waits):
                        out.append(mk_nop(inst.engine, waits[i : i + max_waits]))
                    si.on_wait = waits[len(waits) - max_waits :]
                out.append(inst)
            if changed:
                bb.instructions = out
    return nc


# ---------------------------------------------------------------------------
# device kernel builders
# ---------------------------------------------------------------------------


def build_A(FC=4):
    """u1 = dinv * (x @ W1). xT host layout [128, NT, FC, 128] bf16."""
    nc = bass.Bass()
    xT = nc.dram_tensor("xT", [128, NT, FC, 128], BF16, kind="ExternalInput")
    W1b = nc.dram_tensor("W1b", [128, FC, 16], BF16, kind="ExternalInput")
    dinva = nc.dram_tensor("dinva", [128, NT], F32, kind="ExternalInput")
    u1 = nc.dram_tensor("u1", [128, NT, 16], BF16, kind="ExternalOutput")
    TB = 8  # node-tiles per DMA batch (8KB per partition)
    with tile.TileContext(nc) as tc:
        with (
            tc.tile_pool(name="sbuf", bufs=3) as pool,
            tc.tile_pool(name="stat", bufs=1) as spool,
            tc.tile_pool(name="psum", bufs=8, space="PSUM") as pp,
        ):
            w1 = spool.tile([128, FC, 16], BF16)
            nc.sync.dma_start(out=w1[:], in_=W1b[:])
            da = spool.tile([128, NT], F32)
            nc.scalar.dma_start(out=da[:], in_=dinva[:])
            u1_sb = spool.tile([128, NT, 16], BF16)
            for t0 in range(0, NT, TB):
                tb = min(TB, NT - t0)
                xt = pool.tile([128, TB, FC, 128], BF16, tag="xt")
                eng = nc.sync if (t0 // TB) % 2 == 0 else nc.scalar
                eng.dma_start(out=xt[:, :tb], in_=xT[:, t0 : t0 + tb])
                for i in range(tb):
                    ps = pp.tile([128, 16], F32, tag="hps")
                    for fc in range(FC):
                        nc.tensor.matmul(
                            out=ps[:],
                            lhsT=xt[:, i, fc, :],
                            rhs=w1[:, fc, :],
                            start=(fc == 0),
                            stop=(fc == FC - 1),
                        )
                    t = t0 + i
                    nc.vector.tensor_tensor(
                        out=u1_sb[:, t, :],
                        in0=ps[:],
                        in1=da[:, t : t + 1].to_broadcast([128, 16]),
                        op=ALU.mult,
                    )
            nc.sync.dma_start(out=u1[:], in_=u1_sb[:])
    return _legalize_waits(nc)


def _emit_segsum(
    nc, pool, pp, g, dstid_ov_sb, iota_sb, id_sb, blocks_nov, CH, CHOV, on_stripe
):
    """Per-block psum scatter + subslot reduce, delivered in 4-block stripes.

    Per block: T_ID identity chunks (lhsT = id_sb) + blocks_nov[b] overflow
    chunks (lhsT = is_equal one-hot from dstid_ov). All chunks of a block
    accumulate into one PSUM sub-tile; 4 blocks share a bank. After each
    stripe's DVE reduce, on_stripe(b0, nb, s4) consumes the [128, nb, 16]
    f32 stripe so the epilogue overlaps the remaining scatter."""
    g_sc = None
    s_ov = None
    P4 = None
    q = 0
    jov = 0
    NB = len(blocks_nov)
    for b, nov in enumerate(blocks_nov):
        if b % 4 == 0:
            P4 = pp.tile([128, 4, 16, PACK], F32, tag="pblk")
        nch = T_ID + nov
        for k in range(nch):
            if q % SC == 0:
                wsc = min(SC, CH - q)
                g_sc = pool.tile([128, SC, 16, PACK], BF16, tag="gsc")
                eng = nc.sync if (q // SC) % 2 == 0 else nc.scalar
                eng.dma_start(out=g_sc[:, :wsc], in_=g[:, q : q + wsc])
            if k >= T_ID:
                if jov % SCOV == 0:
                    wov = min(SCOV, CHOV - jov)
                    s_ov = pool.tile([128, SCOV, 128], BF16, tag="sov")
                    nc.vector.tensor_tensor(
                        out=s_ov[:, :wov, :],
                        in0=dstid_ov_sb[:, jov : jov + wov].to_broadcast(
                            [128, wov, 128]
                        ),
                        in1=iota_sb[:, :wov, :],
                        op=ALU.is_equal,
                    )
                lhsT = s_ov[:, jov % SCOV, :]
                jov += 1
            else:
                lhsT = id_sb[:]
            nc.tensor.matmul(
                out=P4[:, b % 4],
                lhsT=lhsT,
                rhs=g_sc[:, q % SC],
                start=(k == 0),
                stop=(k == nch - 1),
            )
            q += 1
        if b % 4 == 3 or b == NB - 1:
            b0 = (b // 4) * 4
            nb = b - b0 + 1
            s4 = pool.tile([128, 4, 16], F32, tag="s4")
            nc.vector.tensor_reduce(
                out=s4[:, :nb],
                in_=P4[:, :nb],
                axis=mybir.AxisListType.X,
                op=ALU.add,
            )
            on_stripe(b0, nb, s4)


def build_B(CH, CHOV, blocks_nov):
    """s1 -> agg1 -> relu -> v2 -> t2 = v2 @ W2 (sole output), striped."""
    nc = bass.Bass()
    g = nc.dram_tensor("g", [128, CH, 16, PACK], BF16, kind="ExternalInput")
    dstid = nc.dram_tensor("dstid", [128, CHOV], BF16, kind="ExternalInput")
    iota = nc.dram_tensor("iota", [128, SCOV, 128], BF16, kind="ExternalInput")
    u1own = nc.dram_tensor("u1own", [128, NT, 16], BF16, kind="ExternalInput")
    dinva = nc.dram_tensor("dinva", [128, NT], F32, kind="ExternalInput")
    W2q = nc.dram_tensor("W2q", [64, 4, 16], BF16, kind="ExternalInput")
    identT = nc.dram_tensor("identT", [128, 128], BF16, kind="ExternalInput")
    t2 = nc.dram_tensor("t2", [128, NT, 16], BF16, kind="ExternalOutput")
    with tile.TileContext(nc) as tc:
        with (
            tc.tile_pool(name="sbuf", bufs=3) as pool,
            tc.tile_pool(name="stat", bufs=1) as spool,
            tc.tile_pool(name="psum", bufs=4, space="PSUM") as pp,
            tc.tile_pool(name="psumt", bufs=2, space="PSUM") as ppt,
        ):
            id_sb = spool.tile([128, 128], BF16)
            nc.sync.dma_start(out=id_sb[:], in_=identT[:])
            dstid_sb = spool.tile([128, CHOV], BF16)
            nc.sync.dma_start(out=dstid_sb[:], in_=dstid[:])
            iota_sb = spool.tile([128, SCOV, 128], BF16)
            nc.sync.dma_start(out=iota_sb[:], in_=iota[:])
            u1o_bf = spool.tile([128, NT, 16], BF16)
            nc.scalar.dma_start(out=u1o_bf[:], in_=u1own[:])
            da = spool.tile([128, NT], F32)
            nc.scalar.dma_start(out=da[:], in_=dinva[:])
            w2q_sb = spool.tile([64, 4, 16], BF16)
            nc.scalar.dma_start(out=w2q_sb[:], in_=W2q[:])
            u1o = spool.tile([128, NT, 16], F32)
            nc.scalar.copy(out=u1o[:], in_=u1o_bf[:])
            t2_sb = spool.tile([128, NT, 16], BF16)

            def on_stripe(b0, nb, s4):
                sl = slice(b0, b0 + nb)
                agg = pool.tile([128, 4, 16], F32, tag="agg")
                nc.gpsimd.tensor_tensor(
                    out=agg[:, :nb], in0=s4[:, :nb], in1=u1o[:, sl], op=ALU.add
                )
                nc.gpsimd.tensor_tensor(
                    out=agg[:, :nb], in0=agg[:, :nb],
                    in1=da[:, sl].to_broadcast([128, nb, 16]), op=ALU.mult,
                )
                r4 = pool.tile([128, 4, 16], F32, tag="r4")
                nc.scalar.activation(out=r4[:, :nb], in_=agg[:, :nb], func=AF.Relu)
                v4 = pool.tile([128, 4, 16], BF16, tag="v4")
                nc.vector.tensor_tensor(
                    out=v4[:, :nb], in0=r4[:, :nb],
                    in1=da[:, sl].to_broadcast([128, nb, 16]), op=ALU.mult,
                )
                if nb < 4:
                    nc.vector.memset(v4[:, nb:, :], 0.0)
                trps = ppt.tile([64, 128], BF16, tag="trps")
                nc.tensor.transpose(out=trps[:], in_=v4[:], identity=id_sb[:])
                v2T = pool.tile([64, 128], BF16, tag="v2T")
                nc.scalar.copy(out=v2T[:], in_=trps[:])
                z4 = ppt.tile([128, 4, 16], F32, tag="z4")
                for j in range(nb):
                    nc.tensor.matmul(
                        out=z4[:, j], lhsT=v2T[:], rhs=w2q_sb[:, j, :],
                        start=True, stop=True,
                    )
                nc.scalar.copy(out=t2_sb[:, sl, :], in_=z4[:, :nb])

            _emit_segsum(
                nc, pool, pp, g, dstid_sb, iota_sb, id_sb, blocks_nov, CH, CHOV,
                on_stripe,
            )
            nc.sync.dma_start(out=t2[:], in_=t2_sb[:])
    return _legalize_waits(nc)


def build_C(CH, CHOV, blocks_nov):
    """s2 -> z = dinv*(s2 + t2own) + b2 -> log_softmax, striped."""
    nc = bass.Bass()
    g = nc.dram_tensor("g", [128, CH, 16, PACK], BF16, kind="ExternalInput")
    dstid = nc.dram_tensor("dstid", [128, CHOV], BF16, kind="ExternalInput")
    iota = nc.dram_tensor("iota", [128, SCOV, 128], BF16, kind="ExternalInput")
    t2own = nc.dram_tensor("t2own", [128, NT, 16], BF16, kind="ExternalInput")
    dinva = nc.dram_tensor("dinva", [128, NT], F32, kind="ExternalInput")
    identT = nc.dram_tensor("identT", [128, 128], BF16, kind="ExternalInput")
    outd = nc.dram_tensor("outd", [128, NT, 16], F32, kind="ExternalOutput")
    with tile.TileContext(nc) as tc:
        with (
            tc.tile_pool(name="sbuf", bufs=3) as pool,
            tc.tile_pool(name="stat", bufs=1) as spool,
            tc.tile_pool(name="psum", bufs=6, space="PSUM") as pp,
        ):
            id_sb = spool.tile([128, 128], BF16)
            nc.sync.dma_start(out=id_sb[:], in_=identT[:])
            dstid_sb = spool.tile([128, CHOV], BF16)
            nc.sync.dma_start(out=dstid_sb[:], in_=dstid[:])
            iota_sb = spool.tile([128, SCOV, 128], BF16)
            nc.sync.dma_start(out=iota_sb[:], in_=iota[:])
            t2o_bf = spool.tile([128, NT, 16], BF16)
            nc.scalar.dma_start(out=t2o_bf[:], in_=t2own[:])
            da = spool.tile([128, NT], F32)
            nc.scalar.dma_start(out=da[:], in_=dinva[:])
            t2o = spool.tile([128, NT, 16], F32)
            nc.scalar.copy(out=t2o[:], in_=t2o_bf[:])
            o_sb = spool.tile([128, NT, 16], F32)

            def on_stripe(b0, nb, s4):
                sl = slice(b0, b0 + nb)
                z = pool.tile([128, 4, 16], F32, tag="zs")
                nc.gpsimd.tensor_tensor(
                    out=z[:, :nb], in0=s4[:, :nb], in1=t2o[:, sl], op=ALU.add
                )
                nc.gpsimd.tensor_tensor(
                    out=z[:, :nb], in0=z[:, :nb],
                    in1=da[:, sl].to_broadcast([128, nb, 16]), op=ALU.mult,
                )
                m4 = pool.tile([128, 4], F32, tag="m4")
                nc.vector.tensor_reduce(
                    out=m4[:, :nb], in_=z[:, :nb], axis=mybir.AxisListType.X,
                    op=ALU.max,
                )
                zc = pool.tile([128, 4, 16], F32, tag="zc")
                nc.vector.tensor_tensor(
                    out=zc[:, :nb], in0=z[:, :nb],
                    in1=m4[:, :nb].to_broadcast([128, nb, 16]), op=ALU.subtract,
                )
                e4 = pool.tile([128, 4, 16], F32, tag="e4")
                nc.scalar.activation(out=e4[:, :nb], in_=zc[:, :nb], func=AF.Exp)
                ss = pool.tile([128, 4], F32, tag="ss")
                nc.vector.tensor_reduce(
                    out=ss[:, :nb], in_=e4[:, :nb], axis=mybir.AxisListType.X,
                    op=ALU.add,
                )
                lse = pool.tile([128, 4], F32, tag="lse")
                nc.scalar.activation(out=lse[:, :nb], in_=ss[:, :nb], func=AF.Ln)
                nc.vector.tensor_tensor(
                    out=o_sb[:, sl, :], in0=zc[:, :nb],
                    in1=lse[:, :nb].to_broadcast([128, nb, 16]), op=ALU.subtract,
                )

            _emit_segsum(
                nc, pool, pp, g, dstid_sb, iota_sb, id_sb, blocks_nov, CH, CHOV,
                on_stripe,
            )
            nc.sync.dma_start(out=outd[:], in_=o_sb[:])
    return _legalize_waits(nc)


# ---------------------------------------------------------------------------
# host side
# ---------------------------------------------------------------------------


def _preprocess(edge_index, n_nodes):
    """Sort edges by dst; build the common chunk structure (T_ID identity +
    n_ov overflow chunks per 128-dst block) + per-core slot metadata."""
    src = np.asarray(edge_index[0])
    dst = np.asarray(edge_index[1])
    deg = np.bincount(dst, minlength=n_nodes).astype(np.float32) + 1.0
    dinv = (1.0 / np.sqrt(deg)).astype(np.float32)

    order = np.argsort(dst, kind="stable")
    sdst = dst[order]
    ssrc = src[order]
    bounds = np.searchsorted(sdst, np.arange(N_CORES + 1) * PER_CORE)

    # per-core local in-degree and slot counts
    deg_loc = np.zeros((N_CORES, PADDED), np.int64)
    core_edges = []
    for c in range(N_CORES):
        lo, hi = bounds[c], bounds[c + 1]
        ld = sdst[lo:hi] - c * PER_CORE
        deg_loc[c, : PER_CORE] = np.bincount(ld, minlength=PER_CORE)
        core_edges.append((ld, ssrc[lo:hi]))
    nslots = -(-deg_loc // PACK)                 # [8, PADDED] ceil div
    ovslots = np.maximum(nslots - T_ID, 0)       # [8, PADDED]

    # common structure: overflow chunk count per block = max over cores
    ov_per_block = ovslots.reshape(N_CORES, NT, 128).sum(axis=2)  # [8, NT]
    n_ov = -(-ov_per_block.max(axis=0) // 128)   # [NT]
    blocks_nov = tuple(int(v) for v in n_ov)
    chunk_base = np.concatenate([[0], np.cumsum(T_ID + n_ov)])    # [NT+1]
    CH = int(chunk_base[-1])
    ov_idx_base = np.concatenate([[0], np.cumsum(n_ov)])          # [NT+1]
    CHOV = max(int(ov_idx_base[-1]), 1)

    sent = N_CORES * PADDED  # sentinel row (zeros) in gather tables
    dstid_arrs, sidx_arrs = [], []
    blk_of_dst = np.arange(PADDED) >> 7
    for c in range(N_CORES):
        ov = ovslots[c]
        # exclusive cumsum of overflow slots within each block
        ovc = np.cumsum(ov) - ov
        blk_start = blk_of_dst << 7
        ovbase = ovc - ovc[blk_start]            # [PADDED]
        ld, esrc = core_edges[c]
        gstart = np.concatenate([[0], np.cumsum(deg_loc[c])])
        rank = np.arange(len(ld)) - gstart[ld]
        k_e = rank // PACK
        c_e = rank % PACK
        blk = ld >> 7
        is_id = k_e < T_ID
        q_id = chunk_base[blk] + k_e
        p_id = ld & 127
        ovpos = ovbase[ld] + (k_e - T_ID)
        q_ov = chunk_base[blk] + T_ID + ovpos // 128
        p_ov = ovpos % 128
        q_e = np.where(is_id, q_id, q_ov)
        p_e = np.where(is_id, p_id, p_ov)
        # gather row index: src node -> (core, p, t) -> core*PADDED + p*NT + t
        sc_, rr = esrc // PER_CORE, esrc % PER_CORE
        grow = sc_ * PADDED + (rr % 128) * NT + rr // 128
        sidx = np.full((128, CH, PACK), sent, np.int64)
        sidx[p_e, q_e, c_e] = grow
        dstid = np.full((128, CHOV), -1.0, np.float32)
        m = (~is_id) & (c_e == 0)
        qovc = ov_idx_base[blk[m]] + ovpos[m] // 128
        dstid[p_ov[m], qovc] = (ld[m] & 127).astype(np.float32)
        dstid_arrs.append(dstid.astype(NPBF16))
        sidx_arrs.append(sidx)
    return dinv, CH, CHOV, blocks_nov, dstid_arrs, sidx_arrs


_CACHE = {}
LAST_HW_NS = None
LAST_TIMES = {}


def _record(tag, res, t_wall):
    global LAST_HW_NS
    LAST_TIMES[tag] = t_wall
    if res.exec_time_ns is not None:
        LAST_HW_NS = (LAST_HW_NS or 0) + res.exec_time_ns


def _gather_g(table, sidx):
    """table [8*PADDED+1, 16] bf16, sidx [128, CH, PACK] -> [128, CH, 16, PACK]."""
    vals = table[sidx]  # [128, CH, PACK, 16]
    return np.ascontiguousarray(vals.transpose(0, 1, 3, 2))


def kernel(x, W1, b1, W2, b2, edge_index):
    global LAST_HW_NS
    LAST_HW_NS = None
    LAST_TIMES.clear()
    import time as _time

    x = np.asarray(x, dtype=np.float32)
    W1 = np.asarray(W1, dtype=np.float32)
    b1 = np.asarray(b1, dtype=np.float32)
    W2 = np.asarray(W2, dtype=np.float32)
    b2 = np.asarray(b2, dtype=np.float32)
    edge_index = np.asarray(edge_index)
    n_nodes, fin = x.shape
    FC = fin // 128

    t0 = _time.time()
    dinv, CH, CHOV, blocks_nov, dstid_arrs, sidx_arrs = _preprocess(
        edge_index, n_nodes
    )
    LAST_TIMES["preprocess"] = _time.time() - t0

    key = (n_nodes, CH, CHOV, blocks_nov)
    if key not in _CACHE:
        _CACHE[key] = (
            build_A(FC),
            build_B(CH, CHOV, blocks_nov),
            build_C(CH, CHOV, blocks_nov),
        )
    ncA, ncB, ncC = _CACHE[key]
    cores = list(range(N_CORES))

    # ---- static per-core arrays ----
    t0 = _time.time()
    W1r = np.ascontiguousarray(
        W1.astype(NPBF16).reshape(FC, 128, 16).transpose(1, 0, 2)
    )
    dinva_c = []
    for c in cores:
        dv = np.ones(PADDED, np.float32)
        dv[:PER_CORE] = dinv[c * PER_CORE : (c + 1) * PER_CORE]
        dinva_c.append(np.ascontiguousarray(dv.reshape(NT, 128).T))
    iota_np = np.ascontiguousarray(
        np.broadcast_to(np.arange(128, dtype=np.float32), (128, SCOV, 128))
    ).astype(NPBF16)
    W2bf = W2.astype(NPBF16)
    rdeg_c = []  # sqrt(deg) per core in [128, NT] layout (1/dinva)
    for c in cores:
        rdeg_c.append((1.0 / dinva_c[c]).astype(np.float32))
    w2q = np.zeros((64, 4, 16), NPBF16)
    for j in range(4):
        w2q[16 * j : 16 * j + 16, j] = W2bf
    ident_np = np.eye(128, dtype=np.float32).astype(NPBF16)

    # ---- dispatch A ----
    in_A = []
    for c in cores:
        xs = x[c * PER_CORE : (c + 1) * PER_CORE]
        xp = np.zeros((PADDED, fin), NPBF16)
        xp[: xs.shape[0]] = xs.astype(NPBF16)
        xTr = np.ascontiguousarray(
            xp.reshape(NT, 128, FC, 128).transpose(3, 0, 2, 1)
        )  # [128 f_lo, NT, FC, 128 n]
        in_A.append({"xT": xTr, "W1b": W1r, "dinva": dinva_c[c]})
    LAST_TIMES["prepA"] = _time.time() - t0
    t0 = _time.time()
    resA = run_bass_kernel_spmd(ncA, in_A, core_ids=cores)
    _record("dispatchA", resA, _time.time() - t0)
    u1s = [resA.results[c]["u1"] for c in cores]  # [128, NT, 16] bf16

    # ---- host gather for layer 1 ----
    t0 = _time.time()
    table1 = np.concatenate(
        [u1s[c].reshape(PADDED, 16) for c in cores] + [np.zeros((1, 16), NPBF16)],
        axis=0,
    )
    in_B = []
    for c in cores:
        # fold the post-norm bias: dinv*(s + u1own + b1*sqrt(deg)) == dinv*(s+u1own) + b1
        u1f = u1s[c].astype(np.float32) + b1[None, None, :] * rdeg_c[c][:, :, None]
        in_B.append(
            {
                "g": _gather_g(table1, sidx_arrs[c]),
                "dstid": dstid_arrs[c],
                "iota": iota_np,
                "u1own": u1f.astype(NPBF16),
                "dinva": dinva_c[c],
                "W2q": w2q,
                "identT": ident_np,
            }
        )
    LAST_TIMES["gather1"] = _time.time() - t0
    t0 = _time.time()
    resB = run_bass_kernel_spmd(ncB, in_B, core_ids=cores)
    _record("dispatchB", resB, _time.time() - t0)
    t2s = [resB.results[c]["t2"] for c in cores]

    # ---- host gather for layer 2 ----
    t0 = _time.time()
    table2 = np.concatenate(
        [t2s[c].reshape(PADDED, 16) for c in cores] + [np.zeros((1, 16), NPBF16)],
        axis=0,
    )
    in_C = []
    for c in cores:
        t2f = t2s[c].astype(np.float32) + b2[None, None, :] * rdeg_c[c][:, :, None]
        in_C.append(
            {
                "g": _gather_g(table2, sidx_arrs[c]),
                "dstid": dstid_arrs[c],
                "iota": iota_np,
                "t2own": t2f.astype(NPBF16),
                "dinva": dinva_c[c],
                "identT": ident_np,
            }
        )
    LAST_TIMES["gather2"] = _time.time() - t0
    t0 = _time.time()
    resC = run_bass_kernel_spmd(ncC, in_C, core_ids=cores)
    _record("dispatchC", resC, _time.time() - t0)
    out = np.concatenate(
        [
            resC.results[c]["outd"].transpose(1, 0, 2).reshape(PADDED, 16)[:PER_CORE]
            for c in cores
        ],
        axis=0,
    ).astype(np.float32)
    return out


# revision 37
# speedup vs baseline: 3.1437x; 1.0426x over previous
"""GCN (2-layer, PyG GCNConv semantics) on 8 Trainium2 NeuronCores.

Strategy (dst-shard, graph-parallel), v2:
- Nodes sharded contiguously across 8 cores (12500 dsts/core).
- 3 SPMD dispatches:
    A: u1 = dinv * (x @ W1)            (x pre-transposed bf16, 4KB DMA runs)
    B: s1 = segsum(g1); agg1 = dinv*(s1+u1own)+b1; r1 = relu;
       v2 = dinv*r1; t2 = v2 @ W2      (outputs only t2, 0.4MB)
    C: s2 = segsum(g2); z = dinv*(s2+t2own)+b2; out = log_softmax(z)
- Segment-sum: edges packed 8-per-slot by destination; per 128-dst
  block, the first T_ID=4 slots of every dst go to "identity" chunks
  (slot partition == dst row, lhsT = static identity - no one-hot
  work), remaining slots to ~1 "overflow" chunk routed by an is_equal
  one-hot. All chunks of a block accumulate into one PSUM tile
  [128, 16f, 8sub]; one DVE reduce per block sums the 8 subslots.
  Chunk structure is common across cores (max-over-cores sizing) so a
  single SPMD program serves all 8 cores.
- The two per-edge value gathers (u1[src]/t2[src] for 3.2M edges) run
  on the host between dispatches (every on-device gather primitive in
  this toolchain was measured unusable: indirect DMA ~1.6us/row,
  GPSIMD gather ucode unloadable under this walrus build).
"""
import os
import sys
import numpy as np

sys.path.insert(0, "/opt/trn_rl_repo")

try:
    # NTFF profiling glue: the image's antenv lacks axon_hooks, which makes
    # run_bass_kernel_spmd(trace=True) crash. Provide it (and register the
    # ctypes hook) so tracing works when BASS_TRACE is set; harmless if not.
    import types as _types

    if "antenv.axon_hooks" not in sys.modules:
        _m = _types.ModuleType("antenv.axon_hooks")
        _st = {}
        _m.set_axon_ntff_profile_hook = lambda h: _st.__setitem__("h", h)
        _m.get_axon_ntff_profile_hook = lambda: _st.get("h")
        sys.modules["antenv.axon_hooks"] = _m
        from trn_agent_boot.trn_boot import _ntff_profile_via_ctypes

        _m.set_axon_ntff_profile_hook(
            _ntff_profile_via_ctypes("/opt/axon/libaxon_pjrt.so")
        )
except Exception:
    pass

import ml_dtypes
import concourse.bass as bass
import concourse.mybir as mybir
import concourse.tile as tile
from concourse.vector_clock import ScopedClock
import concourse.bass_utils as _bu
from concourse.bass_utils import run_bass_kernel_spmd

_orig_upload = _bu.upload_artifacts


def _safe_upload(tmpdir):
    try:
        return _orig_upload(tmpdir)
    except Exception:
        return "local://" + tmpdir


_bu.upload_artifacts = _safe_upload

BF16 = mybir.dt.bfloat16
F32 = mybir.dt.float32
AF = mybir.ActivationFunctionType
ALU = mybir.AluOpType
NPBF16 = ml_dtypes.bfloat16
NPF8 = ml_dtypes.float8_e4m3

G1_FP8 = True    # layer-1 gathered values in fp8 (e4m3)
G2_FP8 = False   # layer-2 gathered values dtype

N_CORES = 8
PER_CORE = 12500
NT = 98              # 128-dst tiles per core (12544 padded)
PADDED = NT * 128
PACK = 8             # edges per slot (matmul N = 16 feats x PACK)
T_ID = 4             # identity chunks per block (slots 0..3 of each dst)
SC = 32              # chunks per g superchunk (DMA batch)
SCOV = 16            # overflow chunks per is_equal batch

# ---------------------------------------------------------------------------
# walrus workaround: only ONE sync-wait command per instruction is accepted.
# ---------------------------------------------------------------------------


def _patched_drain_and_barrier(self, tick_clock, wait_clock):
    nc = self.nc
    carrier = nc.sync.nop(nofuse=True, hint="drain_wait_carrier")
    wait_clock.add_sem_waits(carrier.ins, ScopedClock({None: tick_clock.global_clock}))
    si = carrier.ins.sync_info
    waits = list(si.on_wait or []) if si else []
    if len(waits) > 1:
        si.on_wait = waits[:1]
        for i in range(1, len(waits)):
            extra = nc.sync.nop(nofuse=True, hint="drain_wait_carrier")
            extra.ins.sync_info = mybir.SyncInfo(on_wait=waits[i : i + 1], on_update=[])
    nc.sync.drain()
    nc.all_engine_barrier()
    assert self.sems is not None
    popped = nc._tile_sem_poison_stack.pop()
    assert popped is self._sem_poison
    nc.clear_and_free_semaphores(list(self.sems.allocated().values()))
    nc.all_engine_barrier()


tile.TileContext._drain_and_barrier = _patched_drain_and_barrier


def _legalize_waits(nc, max_waits=1):
    n = [0]

    def mk_nop(engine, waits):
        n[0] += 1
        return mybir.InstNoOp(
            name=f"waitnop-{n[0]}",
            engine=engine,
            ins=[],
            outs=[],
            sync_info=mybir.SyncInfo(on_wait=list(waits), on_update=[]),
            text_hint="wait_carrier",
        )

    for f in nc.m.functions:
        for bb in f.blocks:
            out = []
            changed = False
            for inst in bb.instructions:
                si = inst.sync_info
                waits = list(si.on_wait or []) if si else []
                if len(waits) > max_waits:
                    changed = True
                    for i in range(0, len(waits) - max_waits, max_waits):
                        out.append(mk_nop(inst.engine, waits[i : i + max_waits]))
                    si.on_wait = waits[len(waits) - max_waits :]
                out.append(inst)
            if changed:
                bb.instructions = out
    return nc


# ---------------------------------------------------------------------------
# device kernel builders
# ---------------------------------------------------------------------------


def build_A(FC=4):
    """u1 = dinv * (x @ W1). xT host layout [128, NT, FC, 128] bf16."""
    nc = bass.Bass()
    xT = nc.dram_tensor("xT", [128, NT, FC, 128], BF16, kind="ExternalInput")
    W1b = nc.dram_tensor("W1b", [128, FC, 16], BF16, kind="ExternalInput")
    dinva = nc.dram_tensor("dinva", [128, NT], F32, kind="ExternalInput")
    u1 = nc.dram_tensor("u1", [128, NT, 16], BF16, kind="ExternalOutput")
    TB = 8  # node-tiles per DMA batch (8KB per partition)
    with tile.TileContext(nc) as tc:
        with (
            tc.tile_pool(name="sbuf", bufs=3) as pool,
            tc.tile_pool(name="stat", bufs=1) as spool,
            tc.tile_pool(name="psum", bufs=8, space="PSUM") as pp,
        ):
            w1 = spool.tile([128, FC, 16], BF16)
            nc.sync.dma_start(out=w1[:], in_=W1b[:])
            da = spool.tile([128, NT], F32)
            nc.scalar.dma_start(out=da[:], in_=dinva[:])
            u1_sb = spool.tile([128, NT, 16], BF16)
            batches = [(0, 2), (2, 6)]
            while batches[-1][0] + batches[-1][1] < NT:
                s = batches[-1][0] + batches[-1][1]
                batches.append((s, min(TB, NT - s)))
            for bi, (t0, tb) in enumerate(batches):
                xt = pool.tile([128, TB, FC, 128], BF16, tag="xt")
                eng = nc.sync if bi % 2 == 0 else nc.scalar
                eng.dma_start(out=xt[:, :tb], in_=xT[:, t0 : t0 + tb])
                for i in range(tb):
                    ps = pp.tile([128, 16], F32, tag="hps")
                    for fc in range(FC):
                        nc.tensor.matmul(
                            out=ps[:],
                            lhsT=xt[:, i, fc, :],
                            rhs=w1[:, fc, :],
                            start=(fc == 0),
                            stop=(fc == FC - 1),
                        )
                    t = t0 + i
                    nc.vector.tensor_tensor(
                        out=u1_sb[:, t, :],
                        in0=ps[:],
                        in1=da[:, t : t + 1].to_broadcast([128, 16]),
                        op=ALU.mult,
                    )
            nc.sync.dma_start(out=u1[:], in_=u1_sb[:])
    return _legalize_waits(nc)


def _emit_segsum(
    nc, pool, pp, g, dstid_ov_sb, iota_sb, id_sb, blocks_nov, CH, CHOV, on_stripe,
    pre_hook=None, gdt=BF16,
):
    """Per-block psum scatter + subslot reduce, delivered in 4-block stripes.

    Per block: T_ID identity chunks (lhsT = id_sb) + blocks_nov[b] overflow
    chunks (lhsT = is_equal one-hot from dstid_ov). All chunks of a block
    accumulate into one PSUM sub-tile; 4 blocks share a bank. After each
    stripe's DVE reduce, on_stripe(b0, nb, s4) consumes the [128, nb, 16]
    f32 stripe so the epilogue overlaps the remaining scatter.

    g DMAs use a staged schedule (small first batch so the PE starts fast);
    pre_hook() is emitted right after the first g DMA so secondary input
    loads queue behind it."""
    batches = [(0, 8), (8, 24)]
    while batches[-1][0] + batches[-1][1] < CH:
        s = batches[-1][0] + batches[-1][1]
        batches.append((s, min(SC, CH - s)))
    bi = 0
    batch_end = 0
    g_cur = None
    cur_start = 0
    s_ov = None
    P4 = None
    q = 0
    jov = 0
    NB = len(blocks_nov)
    for b, nov in enumerate(blocks_nov):
        if b % 4 == 0:
            P4 = pp.tile([128, 4, 16, PACK], F32, tag="pblk")
        nch = T_ID + nov
        for k in range(nch):
            if q == batch_end:
                cur_start, w = batches[bi]
                g_cur = pool.tile([128, SC, 16, PACK], gdt, tag="gsc")
                eng = nc.sync if bi % 2 == 0 else nc.scalar
                eng.dma_start(out=g_cur[:, :w], in_=g[:, cur_start : cur_start + w])
                batch_end = cur_start + w
                bi += 1
                if pre_hook is not None:
                    pre_hook()
                    pre_hook = None
            if k >= T_ID:
                if jov % SCOV == 0:
                    wov = min(SCOV, CHOV - jov)
                    s_ov = pool.tile([128, SCOV, 128], BF16, tag="sov")
                    nc.vector.tensor_tensor(
                        out=s_ov[:, :wov, :],
                        in0=dstid_ov_sb[:, jov : jov + wov].to_broadcast(
                            [128, wov, 128]
                        ),
                        in1=iota_sb[:, :wov, :],
                        op=ALU.is_equal,
                    )
                lhsT = s_ov[:, jov % SCOV, :]
                jov += 1
            else:
                lhsT = id_sb[:]
            nc.tensor.matmul(
                out=P4[:, b % 4],
                lhsT=lhsT,
                rhs=g_cur[:, q - cur_start],
                start=(k == 0),
                stop=(k == nch - 1),
            )
            q += 1
        if b % 4 == 3 or b == NB - 1:
            b0 = (b // 4) * 4
            nb = b - b0 + 1
            s4 = pool.tile([128, 4, 16], F32, tag="s4")
            nc.vector.tensor_reduce(
                out=s4[:, :nb],
                in_=P4[:, :nb],
                axis=mybir.AxisListType.X,
                op=ALU.add,
            )
            on_stripe(b0, nb, s4)


def build_B(CH, CHOV, blocks_nov, gdt=BF16):
    """s1 -> agg1 -> relu -> v2 -> t2 = v2 @ W2 (sole output), striped."""
    nc = bass.Bass()
    g = nc.dram_tensor("g", [128, CH, 16, PACK], gdt, kind="ExternalInput")
    dstid = nc.dram_tensor("dstid", [128, CHOV], BF16, kind="ExternalInput")
    iota = nc.dram_tensor("iota", [128, SCOV, 128], BF16, kind="ExternalInput")
    u1own = nc.dram_tensor("u1own", [128, NT, 16], BF16, kind="ExternalInput")
    dinva = nc.dram_tensor("dinva", [128, NT], F32, kind="ExternalInput")
    W2q = nc.dram_tensor("W2q", [64, 4, 16], BF16, kind="ExternalInput")
    identT = nc.dram_tensor("identT", [128, 128], BF16, kind="ExternalInput")
    t2 = nc.dram_tensor("t2", [128, NT, 16], BF16, kind="ExternalOutput")
    with tile.TileContext(nc) as tc:
        with (
            tc.tile_pool(name="sbuf", bufs=3) as pool,
            tc.tile_pool(name="stat", bufs=1) as spool,
            tc.tile_pool(name="psum", bufs=4, space="PSUM") as pp,
            tc.tile_pool(name="psumt", bufs=2, space="PSUM") as ppt,
        ):
            id_sb = spool.tile([128, 128], BF16)
            nc.sync.dma_start(out=id_sb[:], in_=identT[:])
            dstid_sb = spool.tile([128, CHOV], BF16)
            iota_sb = spool.tile([128, SCOV, 128], BF16)
            u1o_bf = spool.tile([128, NT, 16], BF16)
            da = spool.tile([128, NT], F32)
            w2q_sb = spool.tile([64, 4, 16], BF16)
            u1o = spool.tile([128, NT, 16], F32)
            t2_sb = spool.tile([128, NT, 16], BF16)

            def pre_hook():
                nc.sync.dma_start(out=dstid_sb[:], in_=dstid[:])
                nc.sync.dma_start(out=iota_sb[:], in_=iota[:])
                nc.scalar.dma_start(out=u1o_bf[:], in_=u1own[:])
                nc.scalar.dma_start(out=da[:], in_=dinva[:])
                nc.scalar.dma_start(out=w2q_sb[:], in_=W2q[:])
                nc.scalar.copy(out=u1o[:], in_=u1o_bf[:])

            def on_stripe(b0, nb, s4):
                sl = slice(b0, b0 + nb)
                agg = pool.tile([128, 4, 16], F32, tag="agg")
                nc.gpsimd.tensor_tensor(
                    out=agg[:, :nb], in0=s4[:, :nb], in1=u1o[:, sl], op=ALU.add
                )
                nc.gpsimd.tensor_tensor(
                    out=agg[:, :nb], in0=agg[:, :nb],
                    in1=da[:, sl].to_broadcast([128, nb, 16]), op=ALU.mult,
                )
                r4 = pool.tile([128, 4, 16], F32, tag="r4")
                nc.scalar.activation(out=r4[:, :nb], in_=agg[:, :nb], func=AF.Relu)
                v4 = pool.tile([128, 4, 16], BF16, tag="v4")
                nc.vector.tensor_tensor(
                    out=v4[:, :nb], in0=r4[:, :nb],
                    in1=da[:, sl].to_broadcast([128, nb, 16]), op=ALU.mult,
                )
                if nb < 4:
                    nc.vector.memset(v4[:, nb:, :], 0.0)
                trps = ppt.tile([64, 128], BF16, tag="trps")
                nc.tensor.transpose(out=trps[:], in_=v4[:], identity=id_sb[:])
                v2T = pool.tile([64, 128], BF16, tag="v2T")
                nc.scalar.copy(out=v2T[:], in_=trps[:])
                z4 = ppt.tile([128, 4, 16], F32, tag="z4")
                for j in range(nb):
                    nc.tensor.matmul(
                        out=z4[:, j], lhsT=v2T[:], rhs=w2q_sb[:, j, :],
                        start=True, stop=True,
                    )
                nc.scalar.copy(out=t2_sb[:, sl, :], in_=z4[:, :nb])

            _emit_segsum(
                nc, pool, pp, g, dstid_sb, iota_sb, id_sb, blocks_nov, CH, CHOV,
                on_stripe, pre_hook=pre_hook, gdt=gdt,
            )
            nc.sync.dma_start(out=t2[:], in_=t2_sb[:])
    return _legalize_waits(nc)


def build_C(CH, CHOV, blocks_nov, gdt=BF16):
    """s2 -> z = dinv*(s2 + t2own) + b2 -> log_softmax, striped."""
    nc = bass.Bass()
    g = nc.dram_tensor("g", [128, CH, 16, PACK], gdt, kind="ExternalInput")
    dstid = nc.dram_tensor("dstid", [128, CHOV], BF16, kind="ExternalInput")
    iota = nc.dram_tensor("iota", [128, SCOV, 128], BF16, kind="ExternalInput")
    t2own = nc.dram_tensor("t2own", [128, NT, 16], BF16, kind="ExternalInput")
    dinva = nc.dram_tensor("dinva", [128, NT], F32, kind="ExternalInput")
    identT = nc.dram_tensor("identT", [128, 128], BF16, kind="ExternalInput")
    outd = nc.dram_tensor("outd", [128, NT, 16], F32, kind="ExternalOutput")
    with tile.TileContext(nc) as tc:
        with (
            tc.tile_pool(name="sbuf", bufs=3) as pool,
            tc.tile_pool(name="stat", bufs=1) as spool,
            tc.tile_pool(name="psum", bufs=6, space="PSUM") as pp,
        ):
            id_sb = spool.tile([128, 128], BF16)
            nc.sync.dma_start(out=id_sb[:], in_=identT[:])
            dstid_sb = spool.tile([128, CHOV], BF16)
            iota_sb = spool.tile([128, SCOV, 128], BF16)
            t2o_bf = spool.tile([128, NT, 16], BF16)
            da = spool.tile([128, NT], F32)
            t2o = spool.tile([128, NT, 16], F32)
            o_sb = spool.tile([128, NT, 16], F32)

            def pre_hook():
                nc.sync.dma_start(out=dstid_sb[:], in_=dstid[:])
                nc.sync.dma_start(out=iota_sb[:], in_=iota[:])
                nc.scalar.dma_start(out=t2o_bf[:], in_=t2own[:])
                nc.scalar.dma_start(out=da[:], in_=dinva[:])
                nc.scalar.copy(out=t2o[:], in_=t2o_bf[:])

            def on_stripe(b0, nb, s4):
                sl = slice(b0, b0 + nb)
                z = pool.tile([128, 4, 16], F32, tag="zs")
                nc.gpsimd.tensor_tensor(
                    out=z[:, :nb], in0=s4[:, :nb], in1=t2o[:, sl], op=ALU.add
                )
                nc.gpsimd.tensor_tensor(
                    out=z[:, :nb], in0=z[:, :nb],
                    in1=da[:, sl].to_broadcast([128, nb, 16]), op=ALU.mult,
                )
                m4 = pool.tile([128, 4], F32, tag="m4")
                nc.vector.tensor_reduce(
                    out=m4[:, :nb], in_=z[:, :nb], axis=mybir.AxisListType.X,
                    op=ALU.max,
                )
                zc = pool.tile([128, 4, 16], F32, tag="zc")
                nc.vector.tensor_tensor(
                    out=zc[:, :nb], in0=z[:, :nb],
                    in1=m4[:, :nb].to_broadcast([128, nb, 16]), op=ALU.subtract,
                )
                e4 = pool.tile([128, 4, 16], F32, tag="e4")
                nc.scalar.activation(out=e4[:, :nb], in_=zc[:, :nb], func=AF.Exp)
                ss = pool.tile([128, 4], F32, tag="ss")
                nc.vector.tensor_reduce(
                    out=ss[:, :nb], in_=e4[:, :nb], axis=mybir.AxisListType.X,
                    op=ALU.add,
                )
                lse = pool.tile([128, 4], F32, tag="lse")
                nc.scalar.activation(out=lse[:, :nb], in_=ss[:, :nb], func=AF.Ln)
                nc.vector.tensor_tensor(
                    out=o_sb[:, sl, :], in0=zc[:, :nb],
                    in1=lse[:, :nb].to_broadcast([128, nb, 16]), op=ALU.subtract,
                )

            _emit_segsum(
                nc, pool, pp, g, dstid_sb, iota_sb, id_sb, blocks_nov, CH, CHOV,
                on_stripe, pre_hook=pre_hook, gdt=gdt,
            )
            nc.sync.dma_start(out=outd[:], in_=o_sb[:])
    return _legalize_waits(nc)


# ---------------------------------------------------------------------------
# host side
# ---------------------------------------------------------------------------


def _preprocess(edge_index, n_nodes):
    """Sort edges by dst; build the common chunk structure (T_ID identity +
    n_ov overflow chunks per 128-dst block) + per-core slot metadata."""
    src = np.asarray(edge_index[0])
    dst = np.asarray(edge_index[1])
    deg = np.bincount(dst, minlength=n_nodes).astype(np.float32) + 1.0
    dinv = (1.0 / np.sqrt(deg)).astype(np.float32)

    order = np.argsort(dst, kind="stable")
    sdst = dst[order]
    ssrc = src[order]
    bounds = np.searchsorted(sdst, np.arange(N_CORES + 1) * PER_CORE)

    # per-core local in-degree and slot counts
    deg_loc = np.zeros((N_CORES, PADDED), np.int64)
    core_edges = []
    for c in range(N_CORES):
        lo, hi = bounds[c], bounds[c + 1]
        ld = sdst[lo:hi] - c * PER_CORE
        deg_loc[c, : PER_CORE] = np.bincount(ld, minlength=PER_CORE)
        core_edges.append((ld, ssrc[lo:hi]))
    nslots = -(-deg_loc // PACK)                 # [8, PADDED] ceil div
    ovslots = np.maximum(nslots - T_ID, 0)       # [8, PADDED]

    # common structure: overflow chunk count per block = max over cores
    ov_per_block = ovslots.reshape(N_CORES, NT, 128).sum(axis=2)  # [8, NT]
    n_ov = -(-ov_per_block.max(axis=0) // 128)   # [NT]
    blocks_nov = tuple(int(v) for v in n_ov)
    chunk_base = np.concatenate([[0], np.cumsum(T_ID + n_ov)])    # [NT+1]
    CH = int(chunk_base[-1])
    ov_idx_base = np.concatenate([[0], np.cumsum(n_ov)])          # [NT+1]
    CHOV = max(int(ov_idx_base[-1]), 1)

    sent = N_CORES * PADDED  # sentinel row (zeros) in gather tables
    dstid_arrs, sidx_arrs = [], []
    blk_of_dst = np.arange(PADDED) >> 7
    for c in range(N_CORES):
        ov = ovslots[c]
        # exclusive cumsum of overflow slots within each block
        ovc = np.cumsum(ov) - ov
        blk_start = blk_of_dst << 7
        ovbase = ovc - ovc[blk_start]            # [PADDED]
        ld, esrc = core_edges[c]
        gstart = np.concatenate([[0], np.cumsum(deg_loc[c])])
        rank = np.arange(len(ld)) - gstart[ld]
        k_e = rank // PACK
        c_e = rank % PACK
        blk = ld >> 7
        is_id = k_e < T_ID
        q_id = chunk_base[blk] + k_e
        p_id = ld & 127
        ovpos = ovbase[ld] + (k_e - T_ID)
        q_ov = chunk_base[blk] + T_ID + ovpos // 128
        p_ov = ovpos % 128
        q_e = np.where(is_id, q_id, q_ov)
        p_e = np.where(is_id, p_id, p_ov)
        # gather row index: src node -> (core, p, t) -> core*PADDED + p*NT + t
        sc_, rr = esrc // PER_CORE, esrc % PER_CORE
        grow = sc_ * PADDED + (rr % 128) * NT + rr // 128
        sidx = np.full((128, CH, PACK), sent, np.int64)
        sidx[p_e, q_e, c_e] = grow
        dstid = np.full((128, CHOV), -1.0, np.float32)
        m = (~is_id) & (c_e == 0)
        qovc = ov_idx_base[blk[m]] + ovpos[m] // 128
        dstid[p_ov[m], qovc] = (ld[m] & 127).astype(np.float32)
        dstid_arrs.append(dstid.astype(NPBF16))
        sidx_arrs.append(sidx)
    return dinv, CH, CHOV, blocks_nov, dstid_arrs, sidx_arrs


_CACHE = {}
LAST_HW_NS = None
LAST_TIMES = {}


def _record(tag, res, t_wall):
    global LAST_HW_NS
    LAST_TIMES[tag] = t_wall
    if res.exec_time_ns is not None:
        LAST_HW_NS = (LAST_HW_NS or 0) + res.exec_time_ns


def _gather_g(table, sidx):
    """table [8*PADDED+1, 16] bf16, sidx [128, CH, PACK] -> [128, CH, 16, PACK]."""
    vals = table[sidx]  # [128, CH, PACK, 16]
    return np.ascontiguousarray(vals.transpose(0, 1, 3, 2))


def kernel(x, W1, b1, W2, b2, edge_index):
    global LAST_HW_NS
    LAST_HW_NS = None
    LAST_TIMES.clear()
    import time as _time

    x = np.asarray(x, dtype=np.float32)
    W1 = np.asarray(W1, dtype=np.float32)
    b1 = np.asarray(b1, dtype=np.float32)
    W2 = np.asarray(W2, dtype=np.float32)
    b2 = np.asarray(b2, dtype=np.float32)
    edge_index = np.asarray(edge_index)
    n_nodes, fin = x.shape
    FC = fin // 128

    t0 = _time.time()
    dinv, CH, CHOV, blocks_nov, dstid_arrs, sidx_arrs = _preprocess(
        edge_index, n_nodes
    )
    LAST_TIMES["preprocess"] = _time.time() - t0

    key = (n_nodes, CH, CHOV, blocks_nov, G1_FP8, G2_FP8)
    if key not in _CACHE:
        F8 = mybir.dt.float8e4
        _CACHE[key] = (
            build_A(FC),
            build_B(CH, CHOV, blocks_nov, gdt=F8 if G1_FP8 else BF16),
            build_C(CH, CHOV, blocks_nov, gdt=F8 if G2_FP8 else BF16),
        )
    ncA, ncB, ncC = _CACHE[key]
    cores = list(range(N_CORES))

    # ---- static per-core arrays ----
    t0 = _time.time()
    W1r = np.ascontiguousarray(
        W1.astype(NPBF16).reshape(FC, 128, 16).transpose(1, 0, 2)
    )
    dinva_c = []
    for c in cores:
        dv = np.ones(PADDED, np.float32)
        dv[:PER_CORE] = dinv[c * PER_CORE : (c + 1) * PER_CORE]
        dinva_c.append(np.ascontiguousarray(dv.reshape(NT, 128).T))
    iota_np = np.ascontiguousarray(
        np.broadcast_to(np.arange(128, dtype=np.float32), (128, SCOV, 128))
    ).astype(NPBF16)
    W2bf = W2.astype(NPBF16)
    rdeg_c = []  # sqrt(deg) per core in [128, NT] layout (1/dinva)
    for c in cores:
        rdeg_c.append((1.0 / dinva_c[c]).astype(np.float32))
    w2q = np.zeros((64, 4, 16), NPBF16)
    for j in range(4):
        w2q[16 * j : 16 * j + 16, j] = W2bf
    ident_np = np.eye(128, dtype=np.float32).astype(NPBF16)

    # ---- dispatch A ----
    in_A = []
    for c in cores:
        xs = x[c * PER_CORE : (c + 1) * PER_CORE]
        xp = np.zeros((PADDED, fin), NPBF16)
        xp[: xs.shape[0]] = xs.astype(NPBF16)
        xTr = np.ascontiguousarray(
            xp.reshape(NT, 128, FC, 128).transpose(3, 0, 2, 1)
        )  # [128 f_lo, NT, FC, 128 n]
        in_A.append({"xT": xTr, "W1b": W1r, "dinva": dinva_c[c]})
    LAST_TIMES["prepA"] = _time.time() - t0
    t0 = _time.time()
    resA = run_bass_kernel_spmd(ncA, in_A, core_ids=cores)
    _record("dispatchA", resA, _time.time() - t0)
    u1s = [resA.results[c]["u1"] for c in cores]  # [128, NT, 16] bf16

    # ---- host gather for layer 1 ----
    t0 = _time.time()
    table1 = np.concatenate(
        [u1s[c].reshape(PADDED, 16) for c in cores] + [np.zeros((1, 16), NPBF16)],
        axis=0,
    )
    if G1_FP8:
        table1 = table1.astype(NPF8)
    in_B = []
    for c in cores:
        # fold the post-norm bias: dinv*(s + u1own + b1*sqrt(deg)) == dinv*(s+u1own) + b1
        u1f = u1s[c].astype(np.float32) + b1[None, None, :] * rdeg_c[c][:, :, None]
        in_B.append(
            {
                "g": _gather_g(table1, sidx_arrs[c]),
                "dstid": dstid_arrs[c],
                "iota": iota_np,
                "u1own": u1f.astype(NPBF16),
                "dinva": dinva_c[c],
                "W2q": w2q,
                "identT": ident_np,
            }
        )
    LAST_TIMES["gather1"] = _time.time() - t0
    t0 = _time.time()
    resB = run_bass_kernel_spmd(ncB, in_B, core_ids=cores)
    _record("dispatchB", resB, _time.time() - t0)
    t2s = [resB.results[c]["t2"] for c in cores]

    # ---- host gather for layer 2 ----
    t0 = _time.time()
    table2 = np.concatenate(
        [t2s[c].reshape(PADDED, 16) for c in cores] + [np.zeros((1, 16), NPBF16)],
        axis=0,
    )
    if G2_FP8:
        table2 = table2.astype(NPF8)
    in_C = []
    for c in cores:
        t2f = t2s[c].astype(np.float32) + b2[None, None, :] * rdeg_c[c][:, :, None]
        in_C.append(
            {
                "g": _gather_g(table2, sidx_arrs[c]),
                "dstid": dstid_arrs[c],
                "iota": iota_np,
                "t2own": t2f.astype(NPBF16),
                "dinva": dinva_c[c],
                "identT": ident_np,
            }
        )
    LAST_TIMES["gather2"] = _time.time() - t0
    t0 = _time.time()
    resC = run_bass_kernel_spmd(ncC, in_C, core_ids=cores)
    _record("dispatchC", resC, _time.time() - t0)
    out = np.concatenate(
        [
            resC.results[c]["outd"].transpose(1, 0, 2).reshape(PADDED, 16)[:PER_CORE]
            for c in cores
        ],
        axis=0,
    ).astype(np.float32)
    return out


# revision 39
# speedup vs baseline: 3.1923x; 1.0155x over previous
"""GCN (2-layer, PyG GCNConv semantics) on 8 Trainium2 NeuronCores.

Strategy (dst-shard, graph-parallel), v2:
- Nodes sharded contiguously across 8 cores (12500 dsts/core).
- 3 SPMD dispatches:
    A: u1 = dinv * (x @ W1)            (x pre-transposed bf16, 4KB DMA runs)
    B: s1 = segsum(g1); agg1 = dinv*(s1+u1own)+b1; r1 = relu;
       v2 = dinv*r1; t2 = v2 @ W2      (outputs only t2, 0.4MB)
    C: s2 = segsum(g2); z = dinv*(s2+t2own)+b2; out = log_softmax(z)
- Segment-sum: edges packed 8-per-slot by destination; per 128-dst
  block, the first T_ID=4 slots of every dst go to "identity" chunks
  (slot partition == dst row, lhsT = static identity - no one-hot
  work), remaining slots to ~1 "overflow" chunk routed by an is_equal
  one-hot. All chunks of a block accumulate into one PSUM tile
  [128, 16f, 8sub]; one DVE reduce per block sums the 8 subslots.
  Chunk structure is common across cores (max-over-cores sizing) so a
  single SPMD program serves all 8 cores.
- The two per-edge value gathers (u1[src]/t2[src] for 3.2M edges) run
  on the host between dispatches (every on-device gather primitive in
  this toolchain was measured unusable: indirect DMA ~1.6us/row,
  GPSIMD gather ucode unloadable under this walrus build).
"""
import os
import sys
import numpy as np

sys.path.insert(0, "/opt/trn_rl_repo")

try:
    # NTFF profiling glue: the image's antenv lacks axon_hooks, which makes
    # run_bass_kernel_spmd(trace=True) crash. Provide it (and register the
    # ctypes hook) so tracing works when BASS_TRACE is set; harmless if not.
    import types as _types

    if "antenv.axon_hooks" not in sys.modules:
        _m = _types.ModuleType("antenv.axon_hooks")
        _st = {}
        _m.set_axon_ntff_profile_hook = lambda h: _st.__setitem__("h", h)
        _m.get_axon_ntff_profile_hook = lambda: _st.get("h")
        sys.modules["antenv.axon_hooks"] = _m
        from trn_agent_boot.trn_boot import _ntff_profile_via_ctypes

        _m.set_axon_ntff_profile_hook(
            _ntff_profile_via_ctypes("/opt/axon/libaxon_pjrt.so")
        )
except Exception:
    pass

import ml_dtypes
import concourse.bass as bass
import concourse.mybir as mybir
import concourse.tile as tile
from concourse.vector_clock import ScopedClock
import concourse.bass_utils as _bu
from concourse.bass_utils import run_bass_kernel_spmd

_orig_upload = _bu.upload_artifacts


def _safe_upload(tmpdir):
    try:
        return _orig_upload(tmpdir)
    except Exception:
        return "local://" + tmpdir


_bu.upload_artifacts = _safe_upload

BF16 = mybir.dt.bfloat16
F32 = mybir.dt.float32
AF = mybir.ActivationFunctionType
ALU = mybir.AluOpType
NPBF16 = ml_dtypes.bfloat16
NPF8 = ml_dtypes.float8_e4m3

G1_FP8 = True    # layer-1 gathered values in fp8 (e4m3)
G2_FP8 = True    # layer-2 gathered values dtype
X_FP8 = False    # x (dispatch A input) dtype

N_CORES = 8
PER_CORE = 12500
NT = 98              # 128-dst tiles per core (12544 padded)
PADDED = NT * 128
PACK = 8             # edges per slot (matmul N = 16 feats x PACK)
T_ID = 4             # identity chunks per block (slots 0..3 of each dst)
SC = 32              # chunks per g superchunk (DMA batch)
SCOV = 16            # overflow chunks per is_equal batch

# ---------------------------------------------------------------------------
# walrus workaround: only ONE sync-wait command per instruction is accepted.
# ---------------------------------------------------------------------------


def _patched_drain_and_barrier(self, tick_clock, wait_clock):
    nc = self.nc
    carrier = nc.sync.nop(nofuse=True, hint="drain_wait_carrier")
    wait_clock.add_sem_waits(carrier.ins, ScopedClock({None: tick_clock.global_clock}))
    si = carrier.ins.sync_info
    waits = list(si.on_wait or []) if si else []
    if len(waits) > 1:
        si.on_wait = waits[:1]
        for i in range(1, len(waits)):
            extra = nc.sync.nop(nofuse=True, hint="drain_wait_carrier")
            extra.ins.sync_info = mybir.SyncInfo(on_wait=waits[i : i + 1], on_update=[])
    nc.sync.drain()
    nc.all_engine_barrier()
    assert self.sems is not None
    popped = nc._tile_sem_poison_stack.pop()
    assert popped is self._sem_poison
    nc.clear_and_free_semaphores(list(self.sems.allocated().values()))
    nc.all_engine_barrier()


tile.TileContext._drain_and_barrier = _patched_drain_and_barrier


def _legalize_waits(nc, max_waits=1):
    n = [0]

    def mk_nop(engine, waits):
        n[0] += 1
        return mybir.InstNoOp(
            name=f"waitnop-{n[0]}",
            engine=engine,
            ins=[],
            outs=[],
            sync_info=mybir.SyncInfo(on_wait=list(waits), on_update=[]),
            text_hint="wait_carrier",
        )

    for f in nc.m.functions:
        for bb in f.blocks:
            out = []
            changed = False
            for inst in bb.instructions:
                si = inst.sync_info
                waits = list(si.on_wait or []) if si else []
                if len(waits) > max_waits:
                    changed = True
                    for i in range(0, len(waits) - max_waits, max_waits):
                        out.append(mk_nop(inst.engine, waits[i : i + max_waits]))
                    si.on_wait = waits[len(waits) - max_waits :]
                out.append(inst)
            if changed:
                bb.instructions = out
    return nc


# ---------------------------------------------------------------------------
# device kernel builders
# ---------------------------------------------------------------------------


def build_A(FC=4, xdt=BF16):
    """u1 = dinv * (x @ W1). xT host layout [128, NT, FC, 128]."""
    nc = bass.Bass()
    xT = nc.dram_tensor("xT", [128, NT, FC, 128], xdt, kind="ExternalInput")
    W1b = nc.dram_tensor("W1b", [128, FC, 16], BF16, kind="ExternalInput")
    dinva = nc.dram_tensor("dinva", [128, NT], F32, kind="ExternalInput")
    u1 = nc.dram_tensor("u1", [128, NT, 16], BF16, kind="ExternalOutput")
    TB = 8  # node-tiles per DMA batch (8KB per partition)
    with tile.TileContext(nc) as tc:
        with (
            tc.tile_pool(name="sbuf", bufs=3) as pool,
            tc.tile_pool(name="stat", bufs=1) as spool,
            tc.tile_pool(name="psum", bufs=8, space="PSUM") as pp,
        ):
            w1 = spool.tile([128, FC, 16], BF16)
            nc.sync.dma_start(out=w1[:], in_=W1b[:])
            da = spool.tile([128, NT], F32)
            nc.scalar.dma_start(out=da[:], in_=dinva[:])
            u1_sb = spool.tile([128, NT, 16], BF16)
            batches = [(0, 2), (2, 6)]
            while batches[-1][0] + batches[-1][1] < NT:
                s = batches[-1][0] + batches[-1][1]
                batches.append((s, min(TB, NT - s)))
            for bi, (t0, tb) in enumerate(batches):
                xt = pool.tile([128, TB, FC, 128], xdt, tag="xt")
                eng = nc.sync if bi % 2 == 0 else nc.scalar
                eng.dma_start(out=xt[:, :tb], in_=xT[:, t0 : t0 + tb])
                for i in range(tb):
                    ps = pp.tile([128, 16], F32, tag="hps")
                    for fc in range(FC):
                        nc.tensor.matmul(
                            out=ps[:],
                            lhsT=xt[:, i, fc, :],
                            rhs=w1[:, fc, :],
                            start=(fc == 0),
                            stop=(fc == FC - 1),
                        )
                    t = t0 + i
                    nc.vector.tensor_tensor(
                        out=u1_sb[:, t, :],
                        in0=ps[:],
                        in1=da[:, t : t + 1].to_broadcast([128, 16]),
                        op=ALU.mult,
                    )
            nc.sync.dma_start(out=u1[:], in_=u1_sb[:])
    return _legalize_waits(nc)


def _emit_segsum(
    nc, pool, pp, g, oh, id_sb, blocks_nov, CH, CHOV, on_stripe,
    pre_hook=None, gdt=BF16,
):
    """Per-block psum scatter + subslot reduce, delivered in 4-block stripes.

    Per block: T_ID identity chunks (lhsT = id_sb) + blocks_nov[b] overflow
    chunks (lhsT = host-precomputed one-hot slices streamed from `oh`). All
    chunks of a block accumulate into one PSUM sub-tile; 4 blocks share a
    bank. After each stripe's DVE reduce, on_stripe(b0, nb, s4) consumes the
    [128, nb, 16] f32 stripe so the epilogue overlaps the remaining scatter.

    g and oh DMAs use staged schedules (small first batches so the PE starts
    fast); pre_hook() is emitted right after the first g DMA so secondary
    input loads queue behind it."""
    batches = [(0, 8), (8, 24)]
    while batches[-1][0] + batches[-1][1] < CH:
        s = batches[-1][0] + batches[-1][1]
        batches.append((s, min(SC, CH - s)))
    bi = 0
    batch_end = 0
    g_cur = None
    cur_start = 0
    oh_cur = None
    oh_start = 0
    oh_end = 0
    oi = 0
    P4 = None
    q = 0
    jov = 0
    NB = len(blocks_nov)
    for b, nov in enumerate(blocks_nov):
        if b % 4 == 0:
            P4 = pp.tile([128, 4, 16, PACK], F32, tag="pblk")
        nch = T_ID + nov
        for k in range(nch):
            if q == batch_end:
                cur_start, w = batches[bi]
                g_cur = pool.tile([128, SC, 16, PACK], gdt, tag="gsc")
                eng = nc.sync if bi % 2 == 0 else nc.scalar
                eng.dma_start(out=g_cur[:, :w], in_=g[:, cur_start : cur_start + w])
                batch_end = cur_start + w
                bi += 1
                if pre_hook is not None:
                    pre_hook()
                    pre_hook = None
            if k >= T_ID:
                if jov == oh_end:
                    oh_start = jov
                    wov = min(4 if oi == 0 else SCOV, CHOV - jov)
                    oh_cur = pool.tile([128, SCOV, 128], gdt, tag="ohb")
                    eng = nc.scalar if oi % 2 == 0 else nc.sync
                    eng.dma_start(
                        out=oh_cur[:, :wov], in_=oh[:, oh_start : oh_start + wov]
                    )
                    oh_end = oh_start + wov
                    oi += 1
                lhsT = oh_cur[:, jov - oh_start, :]
                jov += 1
            else:
                lhsT = id_sb[:]
            nc.tensor.matmul(
                out=P4[:, b % 4],
                lhsT=lhsT,
                rhs=g_cur[:, q - cur_start],
                start=(k == 0),
                stop=(k == nch - 1),
            )
            q += 1
        if b % 4 == 3 or b == NB - 1:
            b0 = (b // 4) * 4
            nb = b - b0 + 1
            s4 = pool.tile([128, 4, 16], F32, tag="s4")
            nc.vector.tensor_reduce(
                out=s4[:, :nb],
                in_=P4[:, :nb],
                axis=mybir.AxisListType.X,
                op=ALU.add,
            )
            on_stripe(b0, nb, s4)


def build_B(CH, CHOV, blocks_nov, gdt=BF16):
    """s1 -> agg1 -> relu -> v2 -> t2 = v2 @ W2 (sole output), striped."""
    nc = bass.Bass()
    g = nc.dram_tensor("g", [128, CH, 16, PACK], gdt, kind="ExternalInput")
    dstid = nc.dram_tensor("dstid", [128, CHOV], BF16, kind="ExternalInput")
    iota = nc.dram_tensor("iota", [128, SCOV, 128], BF16, kind="ExternalInput")
    u1own = nc.dram_tensor("u1own", [128, NT, 16], BF16, kind="ExternalInput")
    dinva = nc.dram_tensor("dinva", [128, NT], F32, kind="ExternalInput")
    W2q = nc.dram_tensor("W2q", [64, 4, 16], BF16, kind="ExternalInput")
    identT = nc.dram_tensor("identT", [128, 128], BF16, kind="ExternalInput")
    t2 = nc.dram_tensor("t2", [128, NT, 16], BF16, kind="ExternalOutput")
    with tile.TileContext(nc) as tc:
        with (
            tc.tile_pool(name="sbuf", bufs=3) as pool,
            tc.tile_pool(name="stat", bufs=1) as spool,
            tc.tile_pool(name="psum", bufs=4, space="PSUM") as pp,
            tc.tile_pool(name="psumt", bufs=2, space="PSUM") as ppt,
        ):
            id_sb = spool.tile([128, 128], BF16)
            nc.sync.dma_start(out=id_sb[:], in_=identT[:])
            dstid_sb = spool.tile([128, CHOV], BF16)
            iota_sb = spool.tile([128, SCOV, 128], BF16)
            u1o_bf = spool.tile([128, NT, 16], BF16)
            da = spool.tile([128, NT], F32)
            w2q_sb = spool.tile([64, 4, 16], BF16)
            u1o = spool.tile([128, NT, 16], F32)
            t2_sb = spool.tile([128, NT, 16], BF16)

            def pre_hook():
                nc.sync.dma_start(out=dstid_sb[:], in_=dstid[:])
                nc.sync.dma_start(out=iota_sb[:], in_=iota[:])
                nc.scalar.dma_start(out=u1o_bf[:], in_=u1own[:])
                nc.scalar.dma_start(out=da[:], in_=dinva[:])
                nc.scalar.dma_start(out=w2q_sb[:], in_=W2q[:])
                nc.scalar.copy(out=u1o[:], in_=u1o_bf[:])

            def on_stripe(b0, nb, s4):
                sl = slice(b0, b0 + nb)
                agg = pool.tile([128, 4, 16], F32, tag="agg")
                nc.gpsimd.tensor_tensor(
                    out=agg[:, :nb], in0=s4[:, :nb], in1=u1o[:, sl], op=ALU.add
                )
                nc.gpsimd.tensor_tensor(
                    out=agg[:, :nb], in0=agg[:, :nb],
                    in1=da[:, sl].to_broadcast([128, nb, 16]), op=ALU.mult,
                )
                r4 = pool.tile([128, 4, 16], F32, tag="r4")
                nc.scalar.activation(out=r4[:, :nb], in_=agg[:, :nb], func=AF.Relu)
                v4 = pool.tile([128, 4, 16], BF16, tag="v4")
                nc.vector.tensor_tensor(
                    out=v4[:, :nb], in0=r4[:, :nb],
                    in1=da[:, sl].to_broadcast([128, nb, 16]), op=ALU.mult,
                )
                if nb < 4:
                    nc.vector.memset(v4[:, nb:, :], 0.0)
                trps = ppt.tile([64, 128], BF16, tag="trps")
                nc.tensor.transpose(out=trps[:], in_=v4[:], identity=id_sb[:])
                v2T = pool.tile([64, 128], BF16, tag="v2T")
                nc.scalar.copy(out=v2T[:], in_=trps[:])
                z4 = ppt.tile([128, 4, 16], F32, tag="z4")
                for j in range(nb):
                    nc.tensor.matmul(
                        out=z4[:, j], lhsT=v2T[:], rhs=w2q_sb[:, j, :],
                        start=True, stop=True,
                    )
                nc.scalar.copy(out=t2_sb[:, sl, :], in_=z4[:, :nb])

            _emit_segsum(
                nc, pool, pp, g, dstid_sb, iota_sb, id_sb, blocks_nov, CH, CHOV,
                on_stripe, pre_hook=pre_hook, gdt=gdt,
            )
            nc.sync.dma_start(out=t2[:], in_=t2_sb[:])
    return _legalize_waits(nc)


def build_C(CH, CHOV, blocks_nov, gdt=BF16):
    """s2 -> z = dinv*(s2 + t2own) + b2 -> log_softmax, striped."""
    nc = bass.Bass()
    g = nc.dram_tensor("g", [128, CH, 16, PACK], gdt, kind="ExternalInput")
    dstid = nc.dram_tensor("dstid", [128, CHOV], BF16, kind="ExternalInput")
    iota = nc.dram_tensor("iota", [128, SCOV, 128], BF16, kind="ExternalInput")
    t2own = nc.dram_tensor("t2own", [128, NT, 16], BF16, kind="ExternalInput")
    dinva = nc.dram_tensor("dinva", [128, NT], F32, kind="ExternalInput")
    identT = nc.dram_tensor("identT", [128, 128], BF16, kind="ExternalInput")
    outd = nc.dram_tensor("outd", [128, NT, 16], F32, kind="ExternalOutput")
    with tile.TileContext(nc) as tc:
        with (
            tc.tile_pool(name="sbuf", bufs=3) as pool,
            tc.tile_pool(name="stat", bufs=1) as spool,
            tc.tile_pool(name="psum", bufs=6, space="PSUM") as pp,
        ):
            id_sb = spool.tile([128, 128], BF16)
            nc.sync.dma_start(out=id_sb[:], in_=identT[:])
            dstid_sb = spool.tile([128, CHOV], BF16)
            iota_sb = spool.tile([128, SCOV, 128], BF16)
            t2o_bf = spool.tile([128, NT, 16], BF16)
            da = spool.tile([128, NT], F32)
            t2o = spool.tile([128, NT, 16], F32)
            o_sb = spool.tile([128, NT, 16], F32)

            def pre_hook():
                nc.sync.dma_start(out=dstid_sb[:], in_=dstid[:])
                nc.sync.dma_start(out=iota_sb[:], in_=iota[:])
                nc.scalar.dma_start(out=t2o_bf[:], in_=t2own[:])
                nc.scalar.dma_start(out=da[:], in_=dinva[:])
                nc.scalar.copy(out=t2o[:], in_=t2o_bf[:])

            def on_stripe(b0, nb, s4):
                sl = slice(b0, b0 + nb)
                z = pool.tile([128, 4, 16], F32, tag="zs")
                nc.gpsimd.tensor_tensor(
                    out=z[:, :nb], in0=s4[:, :nb], in1=t2o[:, sl], op=ALU.add
                )
                nc.gpsimd.tensor_tensor(
                    out=z[:, :nb], in0=z[:, :nb],
                    in1=da[:, sl].to_broadcast([128, nb, 16]), op=ALU.mult,
                )
                m4 = pool.tile([128, 4], F32, tag="m4")
                nc.vector.tensor_reduce(
                    out=m4[:, :nb], in_=z[:, :nb], axis=mybir.AxisListType.X,
                    op=ALU.max,
                )
                zc = pool.tile([128, 4, 16], F32, tag="zc")
                nc.vector.tensor_tensor(
                    out=zc[:, :nb], in0=z[:, :nb],
                    in1=m4[:, :nb].to_broadcast([128, nb, 16]), op=ALU.subtract,
                )
                e4 = pool.tile([128, 4, 16], F32, tag="e4")
                nc.scalar.activation(out=e4[:, :nb], in_=zc[:, :nb], func=AF.Exp)
                ss = pool.tile([128, 4], F32, tag="ss")
                nc.vector.tensor_reduce(
                    out=ss[:, :nb], in_=e4[:, :nb], axis=mybir.AxisListType.X,
                    op=ALU.add,
                )
                lse = pool.tile([128, 4], F32, tag="lse")
                nc.scalar.activation(out=lse[:, :nb], in_=ss[:, :nb], func=AF.Ln)
                nc.vector.tensor_tensor(
                    out=o_sb[:, sl, :], in0=zc[:, :nb],
                    in1=lse[:, :nb].to_broadcast([128, nb, 16]), op=ALU.subtract,
                )

            _emit_segsum(
                nc, pool, pp, g, dstid_sb, iota_sb, id_sb, blocks_nov, CH, CHOV,
                on_stripe, pre_hook=pre_hook, gdt=gdt,
            )
            nc.sync.dma_start(out=outd[:], in_=o_sb[:])
    return _legalize_waits(nc)


# ---------------------------------------------------------------------------
# host side
# ---------------------------------------------------------------------------


def _preprocess(edge_index, n_nodes):
    """Sort edges by dst; build the common chunk structure (T_ID identity +
    n_ov overflow chunks per 128-dst block) + per-core slot metadata."""
    src = np.asarray(edge_index[0])
    dst = np.asarray(edge_index[1])
    deg = np.bincount(dst, minlength=n_nodes).astype(np.float32) + 1.0
    dinv = (1.0 / np.sqrt(deg)).astype(np.float32)

    order = np.argsort(dst, kind="stable")
    sdst = dst[order]
    ssrc = src[order]
    bounds = np.searchsorted(sdst, np.arange(N_CORES + 1) * PER_CORE)

    # per-core local in-degree and slot counts
    deg_loc = np.zeros((N_CORES, PADDED), np.int64)
    core_edges = []
    for c in range(N_CORES):
        lo, hi = bounds[c], bounds[c + 1]
        ld = sdst[lo:hi] - c * PER_CORE
        deg_loc[c, : PER_CORE] = np.bincount(ld, minlength=PER_CORE)
        core_edges.append((ld, ssrc[lo:hi]))
    nslots = -(-deg_loc // PACK)                 # [8, PADDED] ceil div
    ovslots = np.maximum(nslots - T_ID, 0)       # [8, PADDED]

    # common structure: overflow chunk count per block = max over cores
    ov_per_block = ovslots.reshape(N_CORES, NT, 128).sum(axis=2)  # [8, NT]
    n_ov = -(-ov_per_block.max(axis=0) // 128)   # [NT]
    blocks_nov = tuple(int(v) for v in n_ov)
    chunk_base = np.concatenate([[0], np.cumsum(T_ID + n_ov)])    # [NT+1]
    CH = int(chunk_base[-1])
    ov_idx_base = np.concatenate([[0], np.cumsum(n_ov)])          # [NT+1]
    CHOV = max(int(ov_idx_base[-1]), 1)

    sent = N_CORES * PADDED  # sentinel row (zeros) in gather tables
    oh_arrs, sidx_arrs = [], []
    blk_of_dst = np.arange(PADDED) >> 7
    for c in range(N_CORES):
        ov = ovslots[c]
        # exclusive cumsum of overflow slots within each block
        ovc = np.cumsum(ov) - ov
        blk_start = blk_of_dst << 7
        ovbase = ovc - ovc[blk_start]            # [PADDED]
        ld, esrc = core_edges[c]
        gstart = np.concatenate([[0], np.cumsum(deg_loc[c])])
        rank = np.arange(len(ld)) - gstart[ld]
        k_e = rank // PACK
        c_e = rank % PACK
        blk = ld >> 7
        is_id = k_e < T_ID
        q_id = chunk_base[blk] + k_e
        p_id = ld & 127
        ovpos = ovbase[ld] + (k_e - T_ID)
        q_ov = chunk_base[blk] + T_ID + ovpos // 128
        p_ov = ovpos % 128
        q_e = np.where(is_id, q_id, q_ov)
        p_e = np.where(is_id, p_id, p_ov)
        # gather row index: src node -> (core, p, t) -> core*PADDED + p*NT + t
        sc_, rr = esrc // PER_CORE, esrc % PER_CORE
        grow = sc_ * PADDED + (rr % 128) * NT + rr // 128
        sidx = np.full((128, CH, PACK), sent, np.int64)
        sidx[p_e, q_e, c_e] = grow
        # precomputed overflow one-hots [128 slot, CHOV, 128 row]
        oh = np.zeros((128, CHOV, 128), np.uint8)
        m = (~is_id) & (c_e == 0)
        qovc = ov_idx_base[blk[m]] + ovpos[m] // 128
        oh[p_ov[m], qovc, ld[m] & 127] = 1
        oh_arrs.append(oh)
        sidx_arrs.append(sidx)
    return dinv, CH, CHOV, blocks_nov, oh_arrs, sidx_arrs


_CACHE = {}
LAST_HW_NS = None
LAST_TIMES = {}


def _record(tag, res, t_wall):
    global LAST_HW_NS
    LAST_TIMES[tag] = t_wall
    if res.exec_time_ns is not None:
        LAST_HW_NS = (LAST_HW_NS or 0) + res.exec_time_ns


def _gather_g(table, sidx):
    """table [8*PADDED+1, 16] bf16, sidx [128, CH, PACK] -> [128, CH, 16, PACK]."""
    vals = table[sidx]  # [128, CH, PACK, 16]
    return np.ascontiguousarray(vals.transpose(0, 1, 3, 2))


def kernel(x, W1, b1, W2, b2, edge_index):
    global LAST_HW_NS
    LAST_HW_NS = None
    LAST_TIMES.clear()
    import time as _time

    x = np.asarray(x, dtype=np.float32)
    W1 = np.asarray(W1, dtype=np.float32)
    b1 = np.asarray(b1, dtype=np.float32)
    W2 = np.asarray(W2, dtype=np.float32)
    b2 = np.asarray(b2, dtype=np.float32)
    edge_index = np.asarray(edge_index)
    n_nodes, fin = x.shape
    FC = fin // 128

    t0 = _time.time()
    dinv, CH, CHOV, blocks_nov, dstid_arrs, sidx_arrs = _preprocess(
        edge_index, n_nodes
    )
    LAST_TIMES["preprocess"] = _time.time() - t0

    key = (n_nodes, CH, CHOV, blocks_nov, G1_FP8, G2_FP8, X_FP8)
    if key not in _CACHE:
        F8 = mybir.dt.float8e4
        _CACHE[key] = (
            build_A(FC, xdt=F8 if X_FP8 else BF16),
            build_B(CH, CHOV, blocks_nov, gdt=F8 if G1_FP8 else BF16),
            build_C(CH, CHOV, blocks_nov, gdt=F8 if G2_FP8 else BF16),
        )
    ncA, ncB, ncC = _CACHE[key]
    cores = list(range(N_CORES))

    # ---- static per-core arrays ----
    t0 = _time.time()
    W1r = np.ascontiguousarray(
        W1.astype(NPBF16).reshape(FC, 128, 16).transpose(1, 0, 2)
    )
    dinva_c = []
    for c in cores:
        dv = np.ones(PADDED, np.float32)
        dv[:PER_CORE] = dinv[c * PER_CORE : (c + 1) * PER_CORE]
        dinva_c.append(np.ascontiguousarray(dv.reshape(NT, 128).T))
    iota_np = np.ascontiguousarray(
        np.broadcast_to(np.arange(128, dtype=np.float32), (128, SCOV, 128))
    ).astype(NPBF16)
    W2bf = W2.astype(NPBF16)
    rdeg_c = []  # sqrt(deg) per core in [128, NT] layout (1/dinva)
    for c in cores:
        rdeg_c.append((1.0 / dinva_c[c]).astype(np.float32))
    w2q = np.zeros((64, 4, 16), NPBF16)
    for j in range(4):
        w2q[16 * j : 16 * j + 16, j] = W2bf
    ident_np = np.eye(128, dtype=np.float32).astype(NPBF16)

    # ---- dispatch A ----
    in_A = []
    xnp = NPF8 if X_FP8 else NPBF16
    for c in cores:
        xs = x[c * PER_CORE : (c + 1) * PER_CORE]
        xp = np.zeros((PADDED, fin), xnp)
        xp[: xs.shape[0]] = xs.astype(xnp)
        xTr = np.ascontiguousarray(
            xp.reshape(NT, 128, FC, 128).transpose(3, 0, 2, 1)
        )  # [128 f_lo, NT, FC, 128 n]
        in_A.append({"xT": xTr, "W1b": W1r, "dinva": dinva_c[c]})
    LAST_TIMES["prepA"] = _time.time() - t0
    t0 = _time.time()
    resA = run_bass_kernel_spmd(ncA, in_A, core_ids=cores)
    _record("dispatchA", resA, _time.time() - t0)
    u1s = [resA.results[c]["u1"] for c in cores]  # [128, NT, 16] bf16

    # ---- host gather for layer 1 ----
    t0 = _time.time()
    table1 = np.concatenate(
        [u1s[c].reshape(PADDED, 16) for c in cores] + [np.zeros((1, 16), NPBF16)],
        axis=0,
    )
    if G1_FP8:
        table1 = table1.astype(NPF8)
    in_B = []
    for c in cores:
        # fold the post-norm bias: dinv*(s + u1own + b1*sqrt(deg)) == dinv*(s+u1own) + b1
        u1f = u1s[c].astype(np.float32) + b1[None, None, :] * rdeg_c[c][:, :, None]
        in_B.append(
            {
                "g": _gather_g(table1, sidx_arrs[c]),
                "dstid": dstid_arrs[c],
                "iota": iota_np,
                "u1own": u1f.astype(NPBF16),
                "dinva": dinva_c[c],
                "W2q": w2q,
                "identT": ident_np,
            }
        )
    LAST_TIMES["gather1"] = _time.time() - t0
    t0 = _time.time()
    resB = run_bass_kernel_spmd(ncB, in_B, core_ids=cores)
    _record("dispatchB", resB, _time.time() - t0)
    t2s = [resB.results[c]["t2"] for c in cores]

    # ---- host gather for layer 2 ----
    t0 = _time.time()
    table2 = np.concatenate(
        [t2s[c].reshape(PADDED, 16) for c in cores] + [np.zeros((1, 16), NPBF16)],
        axis=0,
    )
    if G2_FP8:
        table2 = table2.astype(NPF8)
    in_C = []
    for c in cores:
        t2f = t2s[c].astype(np.float32) + b2[None, None, :] * rdeg_c[c][:, :, None]
        in_C.append(
            {
                "g": _gather_g(table2, sidx_arrs[c]),
                "dstid": dstid_arrs[c],
                "iota": iota_np,
                "t2own": t2f.astype(NPBF16),
                "dinva": dinva_c[c],
                "identT": ident_np,
            }
        )
    LAST_TIMES["gather2"] = _time.time() - t0
    t0 = _time.time()
    resC = run_bass_kernel_spmd(ncC, in_C, core_ids=cores)
    _record("dispatchC", resC, _time.time() - t0)
    out = np.concatenate(
        [
            resC.results[c]["outd"].transpose(1, 0, 2).reshape(PADDED, 16)[:PER_CORE]
            for c in cores
        ],
        axis=0,
    ).astype(np.float32)
    return out


# revision 41
# speedup vs baseline: 3.3558x; 1.0512x over previous
"""GCN (2-layer, PyG GCNConv semantics) on 8 Trainium2 NeuronCores.

Strategy (dst-shard, graph-parallel), v2:
- Nodes sharded contiguously across 8 cores (12500 dsts/core).
- 3 SPMD dispatches:
    A: u1 = dinv * (x @ W1)            (x pre-transposed bf16, 4KB DMA runs)
    B: s1 = segsum(g1); agg1 = dinv*(s1+u1own)+b1; r1 = relu;
       v2 = dinv*r1; t2 = v2 @ W2      (outputs only t2, 0.4MB)
    C: s2 = segsum(g2); z = dinv*(s2+t2own)+b2; out = log_softmax(z)
- Segment-sum: edges packed 8-per-slot by destination; per 128-dst
  block, the first T_ID=4 slots of every dst go to "identity" chunks
  (slot partition == dst row, lhsT = static identity - no one-hot
  work), remaining slots to ~1 "overflow" chunk routed by an is_equal
  one-hot. All chunks of a block accumulate into one PSUM tile
  [128, 16f, 8sub]; one DVE reduce per block sums the 8 subslots.
  Chunk structure is common across cores (max-over-cores sizing) so a
  single SPMD program serves all 8 cores.
- The two per-edge value gathers (u1[src]/t2[src] for 3.2M edges) run
  on the host between dispatches (every on-device gather primitive in
  this toolchain was measured unusable: indirect DMA ~1.6us/row,
  GPSIMD gather ucode unloadable under this walrus build).
"""
import os
import sys
import numpy as np

sys.path.insert(0, "/opt/trn_rl_repo")

try:
    # NTFF profiling glue: the image's antenv lacks axon_hooks, which makes
    # run_bass_kernel_spmd(trace=True) crash. Provide it (and register the
    # ctypes hook) so tracing works when BASS_TRACE is set; harmless if not.
    import types as _types

    if "antenv.axon_hooks" not in sys.modules:
        _m = _types.ModuleType("antenv.axon_hooks")
        _st = {}
        _m.set_axon_ntff_profile_hook = lambda h: _st.__setitem__("h", h)
        _m.get_axon_ntff_profile_hook = lambda: _st.get("h")
        sys.modules["antenv.axon_hooks"] = _m
        from trn_agent_boot.trn_boot import _ntff_profile_via_ctypes

        _m.set_axon_ntff_profile_hook(
            _ntff_profile_via_ctypes("/opt/axon/libaxon_pjrt.so")
        )
except Exception:
    pass

import ml_dtypes
import concourse.bass as bass
import concourse.mybir as mybir
import concourse.tile as tile
from concourse.vector_clock import ScopedClock
import concourse.bass_utils as _bu
from concourse.bass_utils import run_bass_kernel_spmd

_orig_upload = _bu.upload_artifacts


def _safe_upload(tmpdir):
    try:
        return _orig_upload(tmpdir)
    except Exception:
        return "local://" + tmpdir


_bu.upload_artifacts = _safe_upload

BF16 = mybir.dt.bfloat16
F32 = mybir.dt.float32
AF = mybir.ActivationFunctionType
ALU = mybir.AluOpType
NPBF16 = ml_dtypes.bfloat16
NPF8 = ml_dtypes.float8_e4m3

G1_FP8 = True    # layer-1 gathered values in fp8 (e4m3)
G2_FP8 = True    # layer-2 gathered values dtype
X_FP8 = False    # x (dispatch A input) dtype

N_CORES = 8
PER_CORE = 12500
NT = 98              # 128-dst tiles per core (12544 padded)
PADDED = NT * 128
PACK = 8             # edges per slot (matmul N = 16 feats x PACK)
T_ID = 4             # identity chunks per block (slots 0..3 of each dst)
SC = 32              # chunks per g superchunk (DMA batch)
SCOV = 16            # overflow chunks per is_equal batch

# ---------------------------------------------------------------------------
# walrus workaround: only ONE sync-wait command per instruction is accepted.
# ---------------------------------------------------------------------------


def _patched_drain_and_barrier(self, tick_clock, wait_clock):
    nc = self.nc
    carrier = nc.sync.nop(nofuse=True, hint="drain_wait_carrier")
    wait_clock.add_sem_waits(carrier.ins, ScopedClock({None: tick_clock.global_clock}))
    si = carrier.ins.sync_info
    waits = list(si.on_wait or []) if si else []
    if len(waits) > 1:
        si.on_wait = waits[:1]
        for i in range(1, len(waits)):
            extra = nc.sync.nop(nofuse=True, hint="drain_wait_carrier")
            extra.ins.sync_info = mybir.SyncInfo(on_wait=waits[i : i + 1], on_update=[])
    nc.sync.drain()
    nc.all_engine_barrier()
    assert self.sems is not None
    popped = nc._tile_sem_poison_stack.pop()
    assert popped is self._sem_poison
    nc.clear_and_free_semaphores(list(self.sems.allocated().values()))
    nc.all_engine_barrier()


tile.TileContext._drain_and_barrier = _patched_drain_and_barrier


def _legalize_waits(nc, max_waits=1):
    n = [0]

    def mk_nop(engine, waits):
        n[0] += 1
        return mybir.InstNoOp(
            name=f"waitnop-{n[0]}",
            engine=engine,
            ins=[],
            outs=[],
            sync_info=mybir.SyncInfo(on_wait=list(waits), on_update=[]),
            text_hint="wait_carrier",
        )

    for f in nc.m.functions:
        for bb in f.blocks:
            out = []
            changed = False
            for inst in bb.instructions:
                si = inst.sync_info
                waits = list(si.on_wait or []) if si else []
                if len(waits) > max_waits:
                    changed = True
                    for i in range(0, len(waits) - max_waits, max_waits):
                        out.append(mk_nop(inst.engine, waits[i : i + max_waits]))
                    si.on_wait = waits[len(waits) - max_waits :]
                out.append(inst)
            if changed:
                bb.instructions = out
    return nc


# ---------------------------------------------------------------------------
# device kernel builders
# ---------------------------------------------------------------------------


def build_A(FC=4, xdt=BF16):
    """u1 = dinv * (x @ W1). xT host layout [128, NT, FC, 128]."""
    nc = bass.Bass()
    xT = nc.dram_tensor("xT", [128, NT, FC, 128], xdt, kind="ExternalInput")
    W1b = nc.dram_tensor("W1b", [128, FC, 16], BF16, kind="ExternalInput")
    dinva = nc.dram_tensor("dinva", [128, NT], F32, kind="ExternalInput")
    u1 = nc.dram_tensor("u1", [128, NT, 16], BF16, kind="ExternalOutput")
    TB = 8  # node-tiles per DMA batch (8KB per partition)
    with tile.TileContext(nc) as tc:
        with (
            tc.tile_pool(name="sbuf", bufs=3) as pool,
            tc.tile_pool(name="stat", bufs=1) as spool,
            tc.tile_pool(name="psum", bufs=8, space="PSUM") as pp,
        ):
            w1 = spool.tile([128, FC, 16], BF16)
            nc.sync.dma_start(out=w1[:], in_=W1b[:])
            da = spool.tile([128, NT], F32)
            nc.scalar.dma_start(out=da[:], in_=dinva[:])
            u1_sb = spool.tile([128, NT, 16], BF16)
            batches = [(0, 2), (2, 6)]
            while batches[-1][0] + batches[-1][1] < NT:
                s = batches[-1][0] + batches[-1][1]
                batches.append((s, min(TB, NT - s)))
            for bi, (t0, tb) in enumerate(batches):
                xt = pool.tile([128, TB, FC, 128], xdt, tag="xt")
                eng = nc.sync if bi % 2 == 0 else nc.scalar
                eng.dma_start(out=xt[:, :tb], in_=xT[:, t0 : t0 + tb])
                for i in range(tb):
                    ps = pp.tile([128, 16], F32, tag="hps")
                    for fc in range(FC):
                        nc.tensor.matmul(
                            out=ps[:],
                            lhsT=xt[:, i, fc, :],
                            rhs=w1[:, fc, :],
                            start=(fc == 0),
                            stop=(fc == FC - 1),
                        )
                    t = t0 + i
                    nc.vector.tensor_tensor(
                        out=u1_sb[:, t, :],
                        in0=ps[:],
                        in1=da[:, t : t + 1].to_broadcast([128, 16]),
                        op=ALU.mult,
                    )
            nc.sync.dma_start(out=u1[:], in_=u1_sb[:])
    return _legalize_waits(nc)


def _emit_segsum(
    nc, pool, pp, g, oh, id_sb, blocks_nov, CH, CHOV, on_stripe,
    pre_hook=None, gdt=BF16,
):
    """Per-block psum scatter + subslot reduce, delivered in 4-block stripes.

    Per block: T_ID identity chunks (lhsT = id_sb) + blocks_nov[b] overflow
    chunks (lhsT = host-precomputed one-hot slices streamed from `oh`). All
    chunks of a block accumulate into one PSUM sub-tile; 4 blocks share a
    bank. After each stripe's DVE reduce, on_stripe(b0, nb, s4) consumes the
    [128, nb, 16] f32 stripe so the epilogue overlaps the remaining scatter.

    g and oh DMAs use staged schedules (small first batches so the PE starts
    fast); pre_hook() is emitted right after the first g DMA so secondary
    input loads queue behind it."""
    batches = [(0, 8), (8, 24)]
    while batches[-1][0] + batches[-1][1] < CH:
        s = batches[-1][0] + batches[-1][1]
        batches.append((s, min(SC, CH - s)))
    bi = 0
    batch_end = 0
    g_cur = None
    cur_start = 0
    oh_cur = None
    oh_start = 0
    oh_end = 0
    oi = 0
    P4 = None
    q = 0
    jov = 0
    NB = len(blocks_nov)
    for b, nov in enumerate(blocks_nov):
        if b % 4 == 0:
            P4 = pp.tile([128, 4, 16, PACK], F32, tag="pblk")
        nch = T_ID + nov
        for k in range(nch):
            if q == batch_end:
                cur_start, w = batches[bi]
                g_cur = pool.tile([128, SC, 16, PACK], gdt, tag="gsc")
                eng = nc.sync if bi % 2 == 0 else nc.scalar
                eng.dma_start(out=g_cur[:, :w], in_=g[:, cur_start : cur_start + w])
                batch_end = cur_start + w
                bi += 1
                if pre_hook is not None:
                    pre_hook()
                    pre_hook = None
            if k >= T_ID:
                if jov == oh_end:
                    oh_start = jov
                    wov = min(4 if oi == 0 else SCOV, CHOV - jov)
                    oh_cur = pool.tile([128, SCOV, 128], gdt, tag="ohb")
                    eng = nc.scalar if oi % 2 == 0 else nc.sync
                    eng.dma_start(
                        out=oh_cur[:, :wov], in_=oh[:, oh_start : oh_start + wov]
                    )
                    oh_end = oh_start + wov
                    oi += 1
                lhsT = oh_cur[:, jov - oh_start, :]
                jov += 1
            else:
                lhsT = id_sb[:]
            nc.tensor.matmul(
                out=P4[:, b % 4],
                lhsT=lhsT,
                rhs=g_cur[:, q - cur_start],
                start=(k == 0),
                stop=(k == nch - 1),
            )
            q += 1
        if b % 4 == 3 or b == NB - 1:
            b0 = (b // 4) * 4
            nb = b - b0 + 1
            s4 = pool.tile([128, 4, 16], F32, tag="s4")
            nc.vector.tensor_reduce(
                out=s4[:, :nb],
                in_=P4[:, :nb],
                axis=mybir.AxisListType.X,
                op=ALU.add,
            )
            on_stripe(b0, nb, s4)


def build_B(CH, CHOV, blocks_nov, gdt=BF16):
    """s1 -> agg1 -> relu -> v2 -> t2 = v2 @ W2 (sole output), striped."""
    nc = bass.Bass()
    g = nc.dram_tensor("g", [128, CH, 16, PACK], gdt, kind="ExternalInput")
    oh = nc.dram_tensor("oh", [128, CHOV, 128], gdt, kind="ExternalInput")
    u1own = nc.dram_tensor("u1own", [128, NT, 16], BF16, kind="ExternalInput")
    dinva = nc.dram_tensor("dinva", [128, NT], F32, kind="ExternalInput")
    W2q = nc.dram_tensor("W2q", [64, 4, 16], BF16, kind="ExternalInput")
    identT = nc.dram_tensor("identT", [128, 128], BF16, kind="ExternalInput")
    t2 = nc.dram_tensor("t2", [128, NT, 16], BF16, kind="ExternalOutput")
    with tile.TileContext(nc) as tc:
        with (
            tc.tile_pool(name="sbuf", bufs=3) as pool,
            tc.tile_pool(name="stat", bufs=1) as spool,
            tc.tile_pool(name="psum", bufs=6, space="PSUM") as pp,
            tc.tile_pool(name="psumt", bufs=1, space="PSUM") as ppt,
        ):
            id_sb = spool.tile([128, 128], BF16)
            nc.sync.dma_start(out=id_sb[:], in_=identT[:])
            u1o_bf = spool.tile([128, NT, 16], BF16)
            da = spool.tile([128, NT], F32)
            w2q_sb = spool.tile([64, 4, 16], BF16)
            u1o = spool.tile([128, NT, 16], F32)
            t2_sb = spool.tile([128, NT, 16], BF16)

            def pre_hook():
                nc.scalar.dma_start(out=u1o_bf[:], in_=u1own[:])
                nc.scalar.dma_start(out=da[:], in_=dinva[:])
                nc.scalar.dma_start(out=w2q_sb[:], in_=W2q[:])
                nc.scalar.copy(out=u1o[:], in_=u1o_bf[:])

            def on_stripe(b0, nb, s4):
                sl = slice(b0, b0 + nb)
                agg = pool.tile([128, 4, 16], F32, tag="agg")
                nc.gpsimd.tensor_tensor(
                    out=agg[:, :nb], in0=s4[:, :nb], in1=u1o[:, sl], op=ALU.add
                )
                nc.gpsimd.tensor_tensor(
                    out=agg[:, :nb], in0=agg[:, :nb],
                    in1=da[:, sl].to_broadcast([128, nb, 16]), op=ALU.mult,
                )
                r4 = pool.tile([128, 4, 16], F32, tag="r4")
                nc.scalar.activation(out=r4[:, :nb], in_=agg[:, :nb], func=AF.Relu)
                v4 = pool.tile([128, 4, 16], BF16, tag="v4")
                nc.vector.tensor_tensor(
                    out=v4[:, :nb], in0=r4[:, :nb],
                    in1=da[:, sl].to_broadcast([128, nb, 16]), op=ALU.mult,
                )
                if nb < 4:
                    nc.vector.memset(v4[:, nb:, :], 0.0)
                trps = ppt.tile([64, 128], BF16, tag="trps")
                nc.tensor.transpose(out=trps[:], in_=v4[:], identity=id_sb[:])
                v2T = pool.tile([64, 128], BF16, tag="v2T")
                nc.scalar.copy(out=v2T[:], in_=trps[:])
                z4 = ppt.tile([128, 4, 16], F32, tag="z4")
                for j in range(nb):
                    nc.tensor.matmul(
                        out=z4[:, j], lhsT=v2T[:], rhs=w2q_sb[:, j, :],
                        start=True, stop=True,
                    )
                nc.scalar.copy(out=t2_sb[:, sl, :], in_=z4[:, :nb])
                nc.sync.dma_start(out=t2[:, sl, :], in_=t2_sb[:, sl, :])

            _emit_segsum(
                nc, pool, pp, g, oh, id_sb, blocks_nov, CH, CHOV,
                on_stripe, pre_hook=pre_hook, gdt=gdt,
            )
    return _legalize_waits(nc)


def build_C(CH, CHOV, blocks_nov, gdt=BF16):
    """s2 -> z = dinv*(s2 + t2own) + b2 -> log_softmax, striped."""
    nc = bass.Bass()
    g = nc.dram_tensor("g", [128, CH, 16, PACK], gdt, kind="ExternalInput")
    oh = nc.dram_tensor("oh", [128, CHOV, 128], gdt, kind="ExternalInput")
    t2own = nc.dram_tensor("t2own", [128, NT, 16], BF16, kind="ExternalInput")
    dinva = nc.dram_tensor("dinva", [128, NT], F32, kind="ExternalInput")
    identT = nc.dram_tensor("identT", [128, 128], BF16, kind="ExternalInput")
    outd = nc.dram_tensor("outd", [128, NT, 16], F32, kind="ExternalOutput")
    with tile.TileContext(nc) as tc:
        with (
            tc.tile_pool(name="sbuf", bufs=3) as pool,
            tc.tile_pool(name="stat", bufs=1) as spool,
            tc.tile_pool(name="psum", bufs=8, space="PSUM") as pp,
        ):
            id_sb = spool.tile([128, 128], BF16)
            nc.sync.dma_start(out=id_sb[:], in_=identT[:])
            t2o_bf = spool.tile([128, NT, 16], BF16)
            da = spool.tile([128, NT], F32)
            t2o = spool.tile([128, NT, 16], F32)
            o_sb = spool.tile([128, NT, 16], F32)

            def pre_hook():
                nc.scalar.dma_start(out=t2o_bf[:], in_=t2own[:])
                nc.scalar.dma_start(out=da[:], in_=dinva[:])
                nc.scalar.copy(out=t2o[:], in_=t2o_bf[:])

            def on_stripe(b0, nb, s4):
                sl = slice(b0, b0 + nb)
                z = pool.tile([128, 4, 16], F32, tag="zs")
                nc.gpsimd.tensor_tensor(
                    out=z[:, :nb], in0=s4[:, :nb], in1=t2o[:, sl], op=ALU.add
                )
                nc.gpsimd.tensor_tensor(
                    out=z[:, :nb], in0=z[:, :nb],
                    in1=da[:, sl].to_broadcast([128, nb, 16]), op=ALU.mult,
                )
                m4 = pool.tile([128, 4], F32, tag="m4")
                nc.vector.tensor_reduce(
                    out=m4[:, :nb], in_=z[:, :nb], axis=mybir.AxisListType.X,
                    op=ALU.max,
                )
                zc = pool.tile([128, 4, 16], F32, tag="zc")
                nc.vector.tensor_tensor(
                    out=zc[:, :nb], in0=z[:, :nb],
                    in1=m4[:, :nb].to_broadcast([128, nb, 16]), op=ALU.subtract,
                )
                e4 = pool.tile([128, 4, 16], F32, tag="e4")
                nc.scalar.activation(out=e4[:, :nb], in_=zc[:, :nb], func=AF.Exp)
                ss = pool.tile([128, 4], F32, tag="ss")
                nc.vector.tensor_reduce(
                    out=ss[:, :nb], in_=e4[:, :nb], axis=mybir.AxisListType.X,
                    op=ALU.add,
                )
                lse = pool.tile([128, 4], F32, tag="lse")
                nc.scalar.activation(out=lse[:, :nb], in_=ss[:, :nb], func=AF.Ln)
                nc.vector.tensor_tensor(
                    out=o_sb[:, sl, :], in0=zc[:, :nb],
                    in1=lse[:, :nb].to_broadcast([128, nb, 16]), op=ALU.subtract,
                )
                nc.sync.dma_start(out=outd[:, sl, :], in_=o_sb[:, sl, :])

            _emit_segsum(
                nc, pool, pp, g, oh, id_sb, blocks_nov, CH, CHOV,
                on_stripe, pre_hook=pre_hook, gdt=gdt,
            )
    return _legalize_waits(nc)


# ---------------------------------------------------------------------------
# host side
# ---------------------------------------------------------------------------


def _preprocess(edge_index, n_nodes):
    """Sort edges by dst; build the common chunk structure (T_ID identity +
    n_ov overflow chunks per 128-dst block) + per-core slot metadata."""
    src = np.asarray(edge_index[0])
    dst = np.asarray(edge_index[1])
    deg = np.bincount(dst, minlength=n_nodes).astype(np.float32) + 1.0
    dinv = (1.0 / np.sqrt(deg)).astype(np.float32)

    order = np.argsort(dst, kind="stable")
    sdst = dst[order]
    ssrc = src[order]
    bounds = np.searchsorted(sdst, np.arange(N_CORES + 1) * PER_CORE)

    # per-core local in-degree and slot counts
    deg_loc = np.zeros((N_CORES, PADDED), np.int64)
    core_edges = []
    for c in range(N_CORES):
        lo, hi = bounds[c], bounds[c + 1]
        ld = sdst[lo:hi] - c * PER_CORE
        deg_loc[c, : PER_CORE] = np.bincount(ld, minlength=PER_CORE)
        core_edges.append((ld, ssrc[lo:hi]))
    nslots = -(-deg_loc // PACK)                 # [8, PADDED] ceil div
    ovslots = np.maximum(nslots - T_ID, 0)       # [8, PADDED]

    # common structure: overflow chunk count per block = max over cores
    ov_per_block = ovslots.reshape(N_CORES, NT, 128).sum(axis=2)  # [8, NT]
    n_ov = -(-ov_per_block.max(axis=0) // 128)   # [NT]
    blocks_nov = tuple(int(v) for v in n_ov)
    chunk_base = np.concatenate([[0], np.cumsum(T_ID + n_ov)])    # [NT+1]
    CH = int(chunk_base[-1])
    ov_idx_base = np.concatenate([[0], np.cumsum(n_ov)])          # [NT+1]
    CHOV = max(int(ov_idx_base[-1]), 1)

    sent = N_CORES * PADDED  # sentinel row (zeros) in gather tables
    oh_arrs, sidx_arrs = [], []
    blk_of_dst = np.arange(PADDED) >> 7
    for c in range(N_CORES):
        ov = ovslots[c]
        # exclusive cumsum of overflow slots within each block
        ovc = np.cumsum(ov) - ov
        blk_start = blk_of_dst << 7
        ovbase = ovc - ovc[blk_start]            # [PADDED]
        ld, esrc = core_edges[c]
        gstart = np.concatenate([[0], np.cumsum(deg_loc[c])])
        rank = np.arange(len(ld)) - gstart[ld]
        k_e = rank // PACK
        c_e = rank % PACK
        blk = ld >> 7
        is_id = k_e < T_ID
        q_id = chunk_base[blk] + k_e
        p_id = ld & 127
        ovpos = ovbase[ld] + (k_e - T_ID)
        q_ov = chunk_base[blk] + T_ID + ovpos // 128
        p_ov = ovpos % 128
        q_e = np.where(is_id, q_id, q_ov)
        p_e = np.where(is_id, p_id, p_ov)
        # gather row index: src node -> (core, p, t) -> core*PADDED + p*NT + t
        sc_, rr = esrc // PER_CORE, esrc % PER_CORE
        grow = sc_ * PADDED + (rr % 128) * NT + rr // 128
        sidx = np.full((128, CH, PACK), sent, np.int64)
        sidx[p_e, q_e, c_e] = grow
        # precomputed overflow one-hots [128 slot, CHOV, 128 row]
        oh = np.zeros((128, CHOV, 128), np.uint8)
        m = (~is_id) & (c_e == 0)
        qovc = ov_idx_base[blk[m]] + ovpos[m] // 128
        oh[p_ov[m], qovc, ld[m] & 127] = 1
        oh_arrs.append(oh)
        sidx_arrs.append(sidx)
    return dinv, CH, CHOV, blocks_nov, oh_arrs, sidx_arrs


_CACHE = {}
LAST_HW_NS = None
LAST_TIMES = {}


def _record(tag, res, t_wall):
    global LAST_HW_NS
    LAST_TIMES[tag] = t_wall
    if res.exec_time_ns is not None:
        LAST_HW_NS = (LAST_HW_NS or 0) + res.exec_time_ns


def _gather_g(table, sidx):
    """table [8*PADDED+1, 16] bf16, sidx [128, CH, PACK] -> [128, CH, 16, PACK]."""
    vals = table[sidx]  # [128, CH, PACK, 16]
    return np.ascontiguousarray(vals.transpose(0, 1, 3, 2))


def kernel(x, W1, b1, W2, b2, edge_index):
    global LAST_HW_NS
    LAST_HW_NS = None
    LAST_TIMES.clear()
    import time as _time

    x = np.asarray(x, dtype=np.float32)
    W1 = np.asarray(W1, dtype=np.float32)
    b1 = np.asarray(b1, dtype=np.float32)
    W2 = np.asarray(W2, dtype=np.float32)
    b2 = np.asarray(b2, dtype=np.float32)
    edge_index = np.asarray(edge_index)
    n_nodes, fin = x.shape
    FC = fin // 128

    t0 = _time.time()
    dinv, CH, CHOV, blocks_nov, oh_arrs, sidx_arrs = _preprocess(
        edge_index, n_nodes
    )
    LAST_TIMES["preprocess"] = _time.time() - t0

    key = (n_nodes, CH, CHOV, blocks_nov, G1_FP8, G2_FP8, X_FP8)
    if key not in _CACHE:
        F8 = mybir.dt.float8e4
        _CACHE[key] = (
            build_A(FC, xdt=F8 if X_FP8 else BF16),
            build_B(CH, CHOV, blocks_nov, gdt=F8 if G1_FP8 else BF16),
            build_C(CH, CHOV, blocks_nov, gdt=F8 if G2_FP8 else BF16),
        )
    ncA, ncB, ncC = _CACHE[key]
    cores = list(range(N_CORES))

    # ---- static per-core arrays ----
    t0 = _time.time()
    W1r = np.ascontiguousarray(
        W1.astype(NPBF16).reshape(FC, 128, 16).transpose(1, 0, 2)
    )
    dinva_c = []
    for c in cores:
        dv = np.ones(PADDED, np.float32)
        dv[:PER_CORE] = dinv[c * PER_CORE : (c + 1) * PER_CORE]
        dinva_c.append(np.ascontiguousarray(dv.reshape(NT, 128).T))
    oh1_c = [a.astype(NPF8 if G1_FP8 else NPBF16) for a in oh_arrs]
    oh2_c = (
        oh1_c if G1_FP8 == G2_FP8
        else [a.astype(NPF8 if G2_FP8 else NPBF16) for a in oh_arrs]
    )
    W2bf = W2.astype(NPBF16)
    rdeg_c = []  # sqrt(deg) per core in [128, NT] layout (1/dinva)
    for c in cores:
        rdeg_c.append((1.0 / dinva_c[c]).astype(np.float32))
    w2q = np.zeros((64, 4, 16), NPBF16)
    for j in range(4):
        w2q[16 * j : 16 * j + 16, j] = W2bf
    ident_np = np.eye(128, dtype=np.float32).astype(NPBF16)

    # ---- dispatch A ----
    in_A = []
    xnp = NPF8 if X_FP8 else NPBF16
    for c in cores:
        xs = x[c * PER_CORE : (c + 1) * PER_CORE]
        xp = np.zeros((PADDED, fin), xnp)
        xp[: xs.shape[0]] = xs.astype(xnp)
        xTr = np.ascontiguousarray(
            xp.reshape(NT, 128, FC, 128).transpose(3, 0, 2, 1)
        )  # [128 f_lo, NT, FC, 128 n]
        in_A.append({"xT": xTr, "W1b": W1r, "dinva": dinva_c[c]})
    LAST_TIMES["prepA"] = _time.time() - t0
    t0 = _time.time()
    resA = run_bass_kernel_spmd(ncA, in_A, core_ids=cores)
    _record("dispatchA", resA, _time.time() - t0)
    u1s = [resA.results[c]["u1"] for c in cores]  # [128, NT, 16] bf16

    # ---- host gather for layer 1 ----
    t0 = _time.time()
    table1 = np.concatenate(
        [u1s[c].reshape(PADDED, 16) for c in cores] + [np.zeros((1, 16), NPBF16)],
        axis=0,
    )
    if G1_FP8:
        table1 = table1.astype(NPF8)
    in_B = []
    for c in cores:
        # fold the post-norm bias: dinv*(s + u1own + b1*sqrt(deg)) == dinv*(s+u1own) + b1
        u1f = u1s[c].astype(np.float32) + b1[None, None, :] * rdeg_c[c][:, :, None]
        in_B.append(
            {
                "g": _gather_g(table1, sidx_arrs[c]),
                "oh": oh1_c[c],
                "u1own": u1f.astype(NPBF16),
                "dinva": dinva_c[c],
                "W2q": w2q,
                "identT": ident_np,
            }
        )
    LAST_TIMES["gather1"] = _time.time() - t0
    t0 = _time.time()
    resB = run_bass_kernel_spmd(ncB, in_B, core_ids=cores)
    _record("dispatchB", resB, _time.time() - t0)
    t2s = [resB.results[c]["t2"] for c in cores]

    # ---- host gather for layer 2 ----
    t0 = _time.time()
    table2 = np.concatenate(
        [t2s[c].reshape(PADDED, 16) for c in cores] + [np.zeros((1, 16), NPBF16)],
        axis=0,
    )
    if G2_FP8:
        table2 = table2.astype(NPF8)
    in_C = []
    for c in cores:
        t2f = t2s[c].astype(np.float32) + b2[None, None, :] * rdeg_c[c][:, :, None]
        in_C.append(
            {
                "g": _gather_g(table2, sidx_arrs[c]),
                "oh": oh2_c[c],
                "t2own": t2f.astype(NPBF16),
                "dinva": dinva_c[c],
                "identT": ident_np,
            }
        )
    LAST_TIMES["gather2"] = _time.time() - t0
    t0 = _time.time()
    resC = run_bass_kernel_spmd(ncC, in_C, core_ids=cores)
    _record("dispatchC", resC, _time.time() - t0)
    out = np.concatenate(
        [
            resC.results[c]["outd"].transpose(1, 0, 2).reshape(PADDED, 16)[:PER_CORE]
            for c in cores
        ],
        axis=0,
    ).astype(np.float32)
    return out


# revision 42
# speedup vs baseline: 3.7342x; 1.1128x over previous
"""GCN (2-layer, PyG GCNConv semantics) on 8 Trainium2 NeuronCores.

Strategy (dst-shard, graph-parallel), v2:
- Nodes sharded contiguously across 8 cores (12500 dsts/core).
- 3 SPMD dispatches:
    A: u1 = dinv * (x @ W1)            (x pre-transposed bf16, 4KB DMA runs)
    B: s1 = segsum(g1); agg1 = dinv*(s1+u1own)+b1; r1 = relu;
       v2 = dinv*r1; t2 = v2 @ W2      (outputs only t2, 0.4MB)
    C: s2 = segsum(g2); z = dinv*(s2+t2own)+b2; out = log_softmax(z)
- Segment-sum: edges packed 8-per-slot by destination; per 128-dst
  block, the first T_ID=4 slots of every dst go to "identity" chunks
  (slot partition == dst row, lhsT = static identity - no one-hot
  work), remaining slots to ~1 "overflow" chunk routed by an is_equal
  one-hot. All chunks of a block accumulate into one PSUM tile
  [128, 16f, 8sub]; one DVE reduce per block sums the 8 subslots.
  Chunk structure is common across cores (max-over-cores sizing) so a
  single SPMD program serves all 8 cores.
- The two per-edge value gathers (u1[src]/t2[src] for 3.2M edges) run
  on the host between dispatches (every on-device gather primitive in
  this toolchain was measured unusable: indirect DMA ~1.6us/row,
  GPSIMD gather ucode unloadable under this walrus build).
"""
import os
import sys
import numpy as np

sys.path.insert(0, "/opt/trn_rl_repo")

try:
    # NTFF profiling glue: the image's antenv lacks axon_hooks, which makes
    # run_bass_kernel_spmd(trace=True) crash. Provide it (and register the
    # ctypes hook) so tracing works when BASS_TRACE is set; harmless if not.
    import types as _types

    if "antenv.axon_hooks" not in sys.modules:
        _m = _types.ModuleType("antenv.axon_hooks")
        _st = {}
        _m.set_axon_ntff_profile_hook = lambda h: _st.__setitem__("h", h)
        _m.get_axon_ntff_profile_hook = lambda: _st.get("h")
        sys.modules["antenv.axon_hooks"] = _m
        from trn_agent_boot.trn_boot import _ntff_profile_via_ctypes

        _m.set_axon_ntff_profile_hook(
            _ntff_profile_via_ctypes("/opt/axon/libaxon_pjrt.so")
        )
except Exception:
    pass

import ml_dtypes
import concourse.bass as bass
import concourse.mybir as mybir
import concourse.tile as tile
from concourse.vector_clock import ScopedClock
import concourse.bass_utils as _bu
from concourse.bass_utils import run_bass_kernel_spmd

_orig_upload = _bu.upload_artifacts


def _safe_upload(tmpdir):
    try:
        return _orig_upload(tmpdir)
    except Exception:
        return "local://" + tmpdir


_bu.upload_artifacts = _safe_upload

BF16 = mybir.dt.bfloat16
F32 = mybir.dt.float32
AF = mybir.ActivationFunctionType
ALU = mybir.AluOpType
NPBF16 = ml_dtypes.bfloat16
NPF8 = ml_dtypes.float8_e4m3

G1_FP8 = True    # layer-1 gathered values in fp8 (e4m3)
G2_FP8 = True    # layer-2 gathered values dtype
X_FP8 = True     # x (dispatch A input) dtype

N_CORES = 8
PER_CORE = 12500
NT = 98              # 128-dst tiles per core (12544 padded)
PADDED = NT * 128
PACK = 8             # edges per slot (matmul N = 16 feats x PACK)
T_ID = 4             # identity chunks per block (slots 0..3 of each dst)
SC = 32              # chunks per g superchunk (DMA batch)
SCOV = 16            # overflow chunks per is_equal batch

# ---------------------------------------------------------------------------
# walrus workaround: only ONE sync-wait command per instruction is accepted.
# ---------------------------------------------------------------------------


def _patched_drain_and_barrier(self, tick_clock, wait_clock):
    nc = self.nc
    carrier = nc.sync.nop(nofuse=True, hint="drain_wait_carrier")
    wait_clock.add_sem_waits(carrier.ins, ScopedClock({None: tick_clock.global_clock}))
    si = carrier.ins.sync_info
    waits = list(si.on_wait or []) if si else []
    if len(waits) > 1:
        si.on_wait = waits[:1]
        for i in range(1, len(waits)):
            extra = nc.sync.nop(nofuse=True, hint="drain_wait_carrier")
            extra.ins.sync_info = mybir.SyncInfo(on_wait=waits[i : i + 1], on_update=[])
    nc.sync.drain()
    nc.all_engine_barrier()
    assert self.sems is not None
    popped = nc._tile_sem_poison_stack.pop()
    assert popped is self._sem_poison
    nc.clear_and_free_semaphores(list(self.sems.allocated().values()))
    nc.all_engine_barrier()


tile.TileContext._drain_and_barrier = _patched_drain_and_barrier


def _legalize_waits(nc, max_waits=1):
    n = [0]

    def mk_nop(engine, waits):
        n[0] += 1
        return mybir.InstNoOp(
            name=f"waitnop-{n[0]}",
            engine=engine,
            ins=[],
            outs=[],
            sync_info=mybir.SyncInfo(on_wait=list(waits), on_update=[]),
            text_hint="wait_carrier",
        )

    for f in nc.m.functions:
        for bb in f.blocks:
            out = []
            changed = False
            for inst in bb.instructions:
                si = inst.sync_info
                waits = list(si.on_wait or []) if si else []
                if len(waits) > max_waits:
                    changed = True
                    for i in range(0, len(waits) - max_waits, max_waits):
                        out.append(mk_nop(inst.engine, waits[i : i + max_waits]))
                    si.on_wait = waits[len(waits) - max_waits :]
                out.append(inst)
            if changed:
                bb.instructions = out
    return nc


# ---------------------------------------------------------------------------
# device kernel builders
# ---------------------------------------------------------------------------


def build_A(FC=4, xdt=BF16):
    """u1 = dinv * (x @ W1). xT host layout [128, NT, FC, 128]."""
    nc = bass.Bass()
    xT = nc.dram_tensor("xT", [128, NT, FC, 128], xdt, kind="ExternalInput")
    W1b = nc.dram_tensor("W1b", [128, FC, 16], BF16, kind="ExternalInput")
    dinva = nc.dram_tensor("dinva", [128, NT], F32, kind="ExternalInput")
    u1 = nc.dram_tensor("u1", [128, NT, 16], BF16, kind="ExternalOutput")
    TB = 8  # node-tiles per DMA batch (8KB per partition)
    with tile.TileContext(nc) as tc:
        with (
            tc.tile_pool(name="sbuf", bufs=3) as pool,
            tc.tile_pool(name="stat", bufs=1) as spool,
            tc.tile_pool(name="psum", bufs=8, space="PSUM") as pp,
        ):
            w1 = spool.tile([128, FC, 16], BF16)
            nc.sync.dma_start(out=w1[:], in_=W1b[:])
            da = spool.tile([128, NT], F32)
            nc.scalar.dma_start(out=da[:], in_=dinva[:])
            u1_sb = spool.tile([128, NT, 16], BF16)
            batches = [(0, 2), (2, 6)]
            while batches[-1][0] + batches[-1][1] < NT:
                s = batches[-1][0] + batches[-1][1]
                batches.append((s, min(TB, NT - s)))
            for bi, (t0, tb) in enumerate(batches):
                xt = pool.tile([128, TB, FC, 128], xdt, tag="xt")
                eng = nc.sync if bi % 2 == 0 else nc.scalar
                eng.dma_start(out=xt[:, :tb], in_=xT[:, t0 : t0 + tb])
                for i in range(tb):
                    ps = pp.tile([128, 16], F32, tag="hps")
                    for fc in range(FC):
                        nc.tensor.matmul(
                            out=ps[:],
                            lhsT=xt[:, i, fc, :],
                            rhs=w1[:, fc, :],
                            start=(fc == 0),
                            stop=(fc == FC - 1),
                        )
                    t = t0 + i
                    nc.vector.tensor_tensor(
                        out=u1_sb[:, t, :],
                        in0=ps[:],
                        in1=da[:, t : t + 1].to_broadcast([128, 16]),
                        op=ALU.mult,
                    )
            nc.sync.dma_start(out=u1[:], in_=u1_sb[:])
    return _legalize_waits(nc)


def _emit_segsum(
    nc, pool, pp, g, oh, id_sb, blocks_nov, CH, CHOV, on_stripe,
    pre_hook=None, gdt=BF16,
):
    """Per-block psum scatter + subslot reduce, delivered in 4-block stripes.

    Per block: T_ID identity chunks (lhsT = id_sb) + blocks_nov[b] overflow
    chunks (lhsT = host-precomputed one-hot slices streamed from `oh`). All
    chunks of a block accumulate into one PSUM sub-tile; 4 blocks share a
    bank. After each stripe's DVE reduce, on_stripe(b0, nb, s4) consumes the
    [128, nb, 16] f32 stripe so the epilogue overlaps the remaining scatter.

    g and oh DMAs use staged schedules (small first batches so the PE starts
    fast); pre_hook() is emitted right after the first g DMA so secondary
    input loads queue behind it."""
    batches = [(0, 8), (8, 24)]
    while batches[-1][0] + batches[-1][1] < CH:
        s = batches[-1][0] + batches[-1][1]
        batches.append((s, min(SC, CH - s)))
    bi = 0
    batch_end = 0
    g_cur = None
    cur_start = 0
    oh_cur = None
    oh_start = 0
    oh_end = 0
    oi = 0
    P4 = None
    q = 0
    jov = 0
    NB = len(blocks_nov)
    for b, nov in enumerate(blocks_nov):
        if b % 4 == 0:
            P4 = pp.tile([128, 4, 16, PACK], F32, tag="pblk")
        nch = T_ID + nov
        for k in range(nch):
            if q == batch_end:
                cur_start, w = batches[bi]
                g_cur = pool.tile([128, SC, 16, PACK], gdt, tag="gsc")
                eng = nc.sync if bi % 2 == 0 else nc.scalar
                eng.dma_start(out=g_cur[:, :w], in_=g[:, cur_start : cur_start + w])
                batch_end = cur_start + w
                bi += 1
                if pre_hook is not None:
                    pre_hook()
                    pre_hook = None
            if k >= T_ID:
                if jov == oh_end:
                    oh_start = jov
                    wov = min(4 if oi == 0 else SCOV, CHOV - jov)
                    oh_cur = pool.tile([128, SCOV, 128], gdt, tag="ohb")
                    eng = nc.scalar if oi % 2 == 0 else nc.sync
                    eng.dma_start(
                        out=oh_cur[:, :wov], in_=oh[:, oh_start : oh_start + wov]
                    )
                    oh_end = oh_start + wov
                    oi += 1
                lhsT = oh_cur[:, jov - oh_start, :]
                jov += 1
            else:
                lhsT = id_sb[:]
            nc.tensor.matmul(
                out=P4[:, b % 4],
                lhsT=lhsT,
                rhs=g_cur[:, q - cur_start],
                start=(k == 0),
                stop=(k == nch - 1),
            )
            q += 1
        if b % 4 == 3 or b == NB - 1:
            b0 = (b // 4) * 4
            nb = b - b0 + 1
            s4 = pool.tile([128, 4, 16], F32, tag="s4")
            nc.vector.tensor_reduce(
                out=s4[:, :nb],
                in_=P4[:, :nb],
                axis=mybir.AxisListType.X,
                op=ALU.add,
            )
            on_stripe(b0, nb, s4)


def build_B(CH, CHOV, blocks_nov, gdt=BF16):
    """s1 -> agg1 -> relu -> v2 -> t2 = v2 @ W2 (sole output), striped."""
    nc = bass.Bass()
    g = nc.dram_tensor("g", [128, CH, 16, PACK], gdt, kind="ExternalInput")
    oh = nc.dram_tensor("oh", [128, CHOV, 128], gdt, kind="ExternalInput")
    u1own = nc.dram_tensor("u1own", [128, NT, 16], BF16, kind="ExternalInput")
    dinva = nc.dram_tensor("dinva", [128, NT], F32, kind="ExternalInput")
    W2q = nc.dram_tensor("W2q", [64, 4, 16], BF16, kind="ExternalInput")
    identT = nc.dram_tensor("identT", [128, 128], BF16, kind="ExternalInput")
    t2 = nc.dram_tensor("t2", [128, NT, 16], BF16, kind="ExternalOutput")
    with tile.TileContext(nc) as tc:
        with (
            tc.tile_pool(name="sbuf", bufs=3) as pool,
            tc.tile_pool(name="stat", bufs=1) as spool,
            tc.tile_pool(name="psum", bufs=6, space="PSUM") as pp,
            tc.tile_pool(name="psumt", bufs=1, space="PSUM") as ppt,
        ):
            id_sb = spool.tile([128, 128], BF16)
            nc.sync.dma_start(out=id_sb[:], in_=identT[:])
            u1o_bf = spool.tile([128, NT, 16], BF16)
            da = spool.tile([128, NT], F32)
            w2q_sb = spool.tile([64, 4, 16], BF16)
            u1o = spool.tile([128, NT, 16], F32)
            t2_sb = spool.tile([128, NT, 16], BF16)

            def pre_hook():
                nc.scalar.dma_start(out=u1o_bf[:], in_=u1own[:])
                nc.scalar.dma_start(out=da[:], in_=dinva[:])
                nc.scalar.dma_start(out=w2q_sb[:], in_=W2q[:])
                nc.scalar.copy(out=u1o[:], in_=u1o_bf[:])

            def on_stripe(b0, nb, s4):
                sl = slice(b0, b0 + nb)
                agg = pool.tile([128, 4, 16], F32, tag="agg")
                nc.gpsimd.tensor_tensor(
                    out=agg[:, :nb], in0=s4[:, :nb], in1=u1o[:, sl], op=ALU.add
                )
                nc.gpsimd.tensor_tensor(
                    out=agg[:, :nb], in0=agg[:, :nb],
                    in1=da[:, sl].to_broadcast([128, nb, 16]), op=ALU.mult,
                )
                r4 = pool.tile([128, 4, 16], F32, tag="r4")
                nc.scalar.activation(out=r4[:, :nb], in_=agg[:, :nb], func=AF.Relu)
                v4 = pool.tile([128, 4, 16], BF16, tag="v4")
                nc.vector.tensor_tensor(
                    out=v4[:, :nb], in0=r4[:, :nb],
                    in1=da[:, sl].to_broadcast([128, nb, 16]), op=ALU.mult,
                )
                if nb < 4:
                    nc.vector.memset(v4[:, nb:, :], 0.0)
                trps = ppt.tile([64, 128], BF16, tag="trps")
                nc.tensor.transpose(out=trps[:], in_=v4[:], identity=id_sb[:])
                v2T = pool.tile([64, 128], BF16, tag="v2T")
                nc.scalar.copy(out=v2T[:], in_=trps[:])
                z4 = ppt.tile([128, 4, 16], F32, tag="z4")
                for j in range(nb):
                    nc.tensor.matmul(
                        out=z4[:, j], lhsT=v2T[:], rhs=w2q_sb[:, j, :],
                        start=True, stop=True,
                    )
                nc.scalar.copy(out=t2_sb[:, sl, :], in_=z4[:, :nb])
                nc.sync.dma_start(out=t2[:, sl, :], in_=t2_sb[:, sl, :])

            _emit_segsum(
                nc, pool, pp, g, oh, id_sb, blocks_nov, CH, CHOV,
                on_stripe, pre_hook=pre_hook, gdt=gdt,
            )
    return _legalize_waits(nc)


def build_C(CH, CHOV, blocks_nov, gdt=BF16):
    """s2 -> z = dinv*(s2 + t2own) + b2 -> log_softmax, striped."""
    nc = bass.Bass()
    g = nc.dram_tensor("g", [128, CH, 16, PACK], gdt, kind="ExternalInput")
    oh = nc.dram_tensor("oh", [128, CHOV, 128], gdt, kind="ExternalInput")
    t2own = nc.dram_tensor("t2own", [128, NT, 16], BF16, kind="ExternalInput")
    dinva = nc.dram_tensor("dinva", [128, NT], F32, kind="ExternalInput")
    identT = nc.dram_tensor("identT", [128, 128], BF16, kind="ExternalInput")
    outd = nc.dram_tensor("outd", [128, NT, 16], F32, kind="ExternalOutput")
    with tile.TileContext(nc) as tc:
        with (
            tc.tile_pool(name="sbuf", bufs=3) as pool,
            tc.tile_pool(name="stat", bufs=1) as spool,
            tc.tile_pool(name="psum", bufs=8, space="PSUM") as pp,
        ):
            id_sb = spool.tile([128, 128], BF16)
            nc.sync.dma_start(out=id_sb[:], in_=identT[:])
            t2o_bf = spool.tile([128, NT, 16], BF16)
            da = spool.tile([128, NT], F32)
            t2o = spool.tile([128, NT, 16], F32)
            o_sb = spool.tile([128, NT, 16], F32)

            def pre_hook():
                nc.scalar.dma_start(out=t2o_bf[:], in_=t2own[:])
                nc.scalar.dma_start(out=da[:], in_=dinva[:])
                nc.scalar.copy(out=t2o[:], in_=t2o_bf[:])

            def on_stripe(b0, nb, s4):
                sl = slice(b0, b0 + nb)
                z = pool.tile([128, 4, 16], F32, tag="zs")
                nc.gpsimd.tensor_tensor(
                    out=z[:, :nb], in0=s4[:, :nb], in1=t2o[:, sl], op=ALU.add
                )
                nc.gpsimd.tensor_tensor(
                    out=z[:, :nb], in0=z[:, :nb],
                    in1=da[:, sl].to_broadcast([128, nb, 16]), op=ALU.mult,
                )
                m4 = pool.tile([128, 4], F32, tag="m4")
                nc.vector.tensor_reduce(
                    out=m4[:, :nb], in_=z[:, :nb], axis=mybir.AxisListType.X,
                    op=ALU.max,
                )
                zc = pool.tile([128, 4, 16], F32, tag="zc")
                nc.vector.tensor_tensor(
                    out=zc[:, :nb], in0=z[:, :nb],
                    in1=m4[:, :nb].to_broadcast([128, nb, 16]), op=ALU.subtract,
                )
                e4 = pool.tile([128, 4, 16], F32, tag="e4")
                nc.scalar.activation(out=e4[:, :nb], in_=zc[:, :nb], func=AF.Exp)
                ss = pool.tile([128, 4], F32, tag="ss")
                nc.vector.tensor_reduce(
                    out=ss[:, :nb], in_=e4[:, :nb], axis=mybir.AxisListType.X,
                    op=ALU.add,
                )
                lse = pool.tile([128, 4], F32, tag="lse")
                nc.scalar.activation(out=lse[:, :nb], in_=ss[:, :nb], func=AF.Ln)
                nc.vector.tensor_tensor(
                    out=o_sb[:, sl, :], in0=zc[:, :nb],
                    in1=lse[:, :nb].to_broadcast([128, nb, 16]), op=ALU.subtract,
                )
                nc.sync.dma_start(out=outd[:, sl, :], in_=o_sb[:, sl, :])

            _emit_segsum(
                nc, pool, pp, g, oh, id_sb, blocks_nov, CH, CHOV,
                on_stripe, pre_hook=pre_hook, gdt=gdt,
            )
    return _legalize_waits(nc)


# ---------------------------------------------------------------------------
# host side
# ---------------------------------------------------------------------------


def _preprocess(edge_index, n_nodes):
    """Sort edges by dst; build the common chunk structure (T_ID identity +
    n_ov overflow chunks per 128-dst block) + per-core slot metadata."""
    src = np.asarray(edge_index[0])
    dst = np.asarray(edge_index[1])
    deg = np.bincount(dst, minlength=n_nodes).astype(np.float32) + 1.0
    dinv = (1.0 / np.sqrt(deg)).astype(np.float32)

    order = np.argsort(dst, kind="stable")
    sdst = dst[order]
    ssrc = src[order]
    bounds = np.searchsorted(sdst, np.arange(N_CORES + 1) * PER_CORE)

    # per-core local in-degree and slot counts
    deg_loc = np.zeros((N_CORES, PADDED), np.int64)
    core_edges = []
    for c in range(N_CORES):
        lo, hi = bounds[c], bounds[c + 1]
        ld = sdst[lo:hi] - c * PER_CORE
        deg_loc[c, : PER_CORE] = np.bincount(ld, minlength=PER_CORE)
        core_edges.append((ld, ssrc[lo:hi]))
    nslots = -(-deg_loc // PACK)                 # [8, PADDED] ceil div
    ovslots = np.maximum(nslots - T_ID, 0)       # [8, PADDED]

    # common structure: overflow chunk count per block = max over cores
    ov_per_block = ovslots.reshape(N_CORES, NT, 128).sum(axis=2)  # [8, NT]
    n_ov = -(-ov_per_block.max(axis=0) // 128)   # [NT]
    blocks_nov = tuple(int(v) for v in n_ov)
    chunk_base = np.concatenate([[0], np.cumsum(T_ID + n_ov)])    # [NT+1]
    CH = int(chunk_base[-1])
    ov_idx_base = np.concatenate([[0], np.cumsum(n_ov)])          # [NT+1]
    CHOV = max(int(ov_idx_base[-1]), 1)

    sent = N_CORES * PADDED  # sentinel row (zeros) in gather tables
    oh_arrs, sidx_arrs = [], []
    blk_of_dst = np.arange(PADDED) >> 7
    for c in range(N_CORES):
        ov = ovslots[c]
        # exclusive cumsum of overflow slots within each block
        ovc = np.cumsum(ov) - ov
        blk_start = blk_of_dst << 7
        ovbase = ovc - ovc[blk_start]            # [PADDED]
        ld, esrc = core_edges[c]
        gstart = np.concatenate([[0], np.cumsum(deg_loc[c])])
        rank = np.arange(len(ld)) - gstart[ld]
        k_e = rank // PACK
        c_e = rank % PACK
        blk = ld >> 7
        is_id = k_e < T_ID
        q_id = chunk_base[blk] + k_e
        p_id = ld & 127
        ovpos = ovbase[ld] + (k_e - T_ID)
        q_ov = chunk_base[blk] + T_ID + ovpos // 128
        p_ov = ovpos % 128
        q_e = np.where(is_id, q_id, q_ov)
        p_e = np.where(is_id, p_id, p_ov)
        # gather row index: src node -> (core, p, t) -> core*PADDED + p*NT + t
        sc_, rr = esrc // PER_CORE, esrc % PER_CORE
        grow = sc_ * PADDED + (rr % 128) * NT + rr // 128
        sidx = np.full((128, CH, PACK), sent, np.int64)
        sidx[p_e, q_e, c_e] = grow
        # precomputed overflow one-hots [128 slot, CHOV, 128 row]
        oh = np.zeros((128, CHOV, 128), np.uint8)
        m = (~is_id) & (c_e == 0)
        qovc = ov_idx_base[blk[m]] + ovpos[m] // 128
        oh[p_ov[m], qovc, ld[m] & 127] = 1
        oh_arrs.append(oh)
        sidx_arrs.append(sidx)
    return dinv, CH, CHOV, blocks_nov, oh_arrs, sidx_arrs


_CACHE = {}
LAST_HW_NS = None
LAST_TIMES = {}


def _record(tag, res, t_wall):
    global LAST_HW_NS
    LAST_TIMES[tag] = t_wall
    if res.exec_time_ns is not None:
        LAST_HW_NS = (LAST_HW_NS or 0) + res.exec_time_ns


def _gather_g(table, sidx):
    """table [8*PADDED+1, 16] bf16, sidx [128, CH, PACK] -> [128, CH, 16, PACK]."""
    vals = table[sidx]  # [128, CH, PACK, 16]
    return np.ascontiguousarray(vals.transpose(0, 1, 3, 2))


def kernel(x, W1, b1, W2, b2, edge_index):
    global LAST_HW_NS
    LAST_HW_NS = None
    LAST_TIMES.clear()
    import time as _time

    x = np.asarray(x, dtype=np.float32)
    W1 = np.asarray(W1, dtype=np.float32)
    b1 = np.asarray(b1, dtype=np.float32)
    W2 = np.asarray(W2, dtype=np.float32)
    b2 = np.asarray(b2, dtype=np.float32)
    edge_index = np.asarray(edge_index)
    n_nodes, fin = x.shape
    FC = fin // 128

    t0 = _time.time()
    dinv, CH, CHOV, blocks_nov, oh_arrs, sidx_arrs = _preprocess(
        edge_index, n_nodes
    )
    LAST_TIMES["preprocess"] = _time.time() - t0

    key = (n_nodes, CH, CHOV, blocks_nov, G1_FP8, G2_FP8, X_FP8)
    if key not in _CACHE:
        F8 = mybir.dt.float8e4
        _CACHE[key] = (
            build_A(FC, xdt=F8 if X_FP8 else BF16),
            build_B(CH, CHOV, blocks_nov, gdt=F8 if G1_FP8 else BF16),
            build_C(CH, CHOV, blocks_nov, gdt=F8 if G2_FP8 else BF16),
        )
    ncA, ncB, ncC = _CACHE[key]
    cores = list(range(N_CORES))

    # ---- static per-core arrays ----
    t0 = _time.time()
    W1r = np.ascontiguousarray(
        W1.astype(NPBF16).reshape(FC, 128, 16).transpose(1, 0, 2)
    )
    dinva_c = []
    for c in cores:
        dv = np.ones(PADDED, np.float32)
        dv[:PER_CORE] = dinv[c * PER_CORE : (c + 1) * PER_CORE]
        dinva_c.append(np.ascontiguousarray(dv.reshape(NT, 128).T))
    oh1_c = [a.astype(NPF8 if G1_FP8 else NPBF16) for a in oh_arrs]
    oh2_c = (
        oh1_c if G1_FP8 == G2_FP8
        else [a.astype(NPF8 if G2_FP8 else NPBF16) for a in oh_arrs]
    )
    W2bf = W2.astype(NPBF16)
    rdeg_c = []  # sqrt(deg) per core in [128, NT] layout (1/dinva)
    for c in cores:
        rdeg_c.append((1.0 / dinva_c[c]).astype(np.float32))
    w2q = np.zeros((64, 4, 16), NPBF16)
    for j in range(4):
        w2q[16 * j : 16 * j + 16, j] = W2bf
    ident_np = np.eye(128, dtype=np.float32).astype(NPBF16)

    # ---- dispatch A ----
    in_A = []
    xnp = NPF8 if X_FP8 else NPBF16
    for c in cores:
        xs = x[c * PER_CORE : (c + 1) * PER_CORE]
        xp = np.zeros((PADDED, fin), xnp)
        xp[: xs.shape[0]] = xs.astype(xnp)
        xTr = np.ascontiguousarray(
            xp.reshape(NT, 128, FC, 128).transpose(3, 0, 2, 1)
        )  # [128 f_lo, NT, FC, 128 n]
        in_A.append({"xT": xTr, "W1b": W1r, "dinva": dinva_c[c]})
    LAST_TIMES["prepA"] = _time.time() - t0
    t0 = _time.time()
    resA = run_bass_kernel_spmd(ncA, in_A, core_ids=cores)
    _record("dispatchA", resA, _time.time() - t0)
    u1s = [resA.results[c]["u1"] for c in cores]  # [128, NT, 16] bf16

    # ---- host gather for layer 1 ----
    t0 = _time.time()
    table1 = np.concatenate(
        [u1s[c].reshape(PADDED, 16) for c in cores] + [np.zeros((1, 16), NPBF16)],
        axis=0,
    )
    if G1_FP8:
        table1 = table1.astype(NPF8)
    in_B = []
    for c in cores:
        # fold the post-norm bias: dinv*(s + u1own + b1*sqrt(deg)) == dinv*(s+u1own) + b1
        u1f = u1s[c].astype(np.float32) + b1[None, None, :] * rdeg_c[c][:, :, None]
        in_B.append(
            {
                "g": _gather_g(table1, sidx_arrs[c]),
                "oh": oh1_c[c],
                "u1own": u1f.astype(NPBF16),
                "dinva": dinva_c[c],
                "W2q": w2q,
                "identT": ident_np,
            }
        )
    LAST_TIMES["gather1"] = _time.time() - t0
    t0 = _time.time()
    resB = run_bass_kernel_spmd(ncB, in_B, core_ids=cores)
    _record("dispatchB", resB, _time.time() - t0)
    t2s = [resB.results[c]["t2"] for c in cores]

    # ---- host gather for layer 2 ----
    t0 = _time.time()
    table2 = np.concatenate(
        [t2s[c].reshape(PADDED, 16) for c in cores] + [np.zeros((1, 16), NPBF16)],
        axis=0,
    )
    if G2_FP8:
        table2 = table2.astype(NPF8)
    in_C = []
    for c in cores:
        t2f = t2s[c].astype(np.float32) + b2[None, None, :] * rdeg_c[c][:, :, None]
        in_C.append(
            {
                "g": _gather_g(table2, sidx_arrs[c]),
                "oh": oh2_c[c],
                "t2own": t2f.astype(NPBF16),
                "dinva": dinva_c[c],
                "identT": ident_np,
            }
        )
    LAST_TIMES["gather2"] = _time.time() - t0
    t0 = _time.time()
    resC = run_bass_kernel_spmd(ncC, in_C, core_ids=cores)
    _record("dispatchC", resC, _time.time() - t0)
    out = np.concatenate(
        [
            resC.results[c]["outd"].transpose(1, 0, 2).reshape(PADDED, 16)[:PER_CORE]
            for c in cores
        ],
        axis=0,
    ).astype(np.float32)
    return out
